# revision 1
# baseline (speedup 1.0000x reference)
"""AgentImputer Trainium2 kernel.

Contract: kernel(**inputs) takes the FULL unsharded inputs (as produced by
reference.setup_inputs()) and returns the FULL output [64, 40, 2] float32.

Strategy: data-parallel over batch B=64 across 8 NeuronCores (8 batches /
core -> 320 folded LSTM rows per core). Tiny LSTM/GCN weights are
replicated. The 128-step TimeLSTM scan runs feature-major ([hid, row]
tiles) so every matmul contracts along partitions; categorical embeddings
are folded into the input matmul via one-hot rows; biases are folded into
the matmuls via a constant-1 row appended to the h/c state; the per-graph
GCN (shared edge_index) becomes dense [40,40] mean-aggregation matmuls.
Matmul operands use float32r (single-pass fp32 streaming when N>=256).
"""

import sys

import numpy as np

sys.path.insert(0, "/opt/trn_rl_repo")

# ---------------------------------------------------------------- constants
B, W, N, F_IN = 64, 128, 40, 66
HID = 100
NUM_CONT = 64
NCLS_POS, NCLS_TEAM = 16, 9
EMB_POS, EMB_TEAM = 4, 3
NCORES = 8
BL = B // NCORES          # 8 local batch elems per core
R = BL * N                # 320 rows per core; row j = 40*b_local + n
OH_P0 = 66                # one-hot pos cols [66:83)
OH_T0 = 83                # one-hot team cols [83:100) (entries 10..16 pad)
XC = 100                  # xs tile feature columns
G4 = 4 * HID


# ---------------------------------------------------------------- host prep
def _host_weights(inputs):
    f32 = np.float32
    Uall_w = np.asarray(inputs["Uall_w"], f32)       # [400, 71]
    Uall_b = np.asarray(inputs["Uall_b"], f32)       # [400]
    Wall_w = np.asarray(inputs["Wall_w"], f32)       # [400, 100]
    Wall_b = np.asarray(inputs["Wall_b"], f32)       # [400]
    Wd_w = np.asarray(inputs["Wd_w"], f32)           # [100, 100]
    Wd_b = np.asarray(inputs["Wd_b"], f32)           # [100]
    lin_w = np.asarray(inputs["lin_w"], f32)         # [100, 100]
    lin_b = np.asarray(inputs["lin_b"], f32)         # [100]
    emb_pos = np.asarray(inputs["emb_pos"], f32)     # [16, 4]
    emb_team = np.asarray(inputs["emb_team"], f32)   # [9, 3]
    edge_index = np.asarray(inputs["edge_index"]).astype(np.int64)  # [2, E]

    # Input-side weights [100, 400]: rows 0:64 continuous features; rows
    # 64,65 (raw categorical codes riding along in the transposed tile) get
    # zero weights; rows 66:83 / 83:93 are one-hot rows with the embedding
    # tables pre-multiplied in (code 0 == missing -> zero row); 93:100 pad.
    WxT = np.zeros((XC, G4), f32)
    WxT[0:NUM_CONT] = Uall_w[:, 0:NUM_CONT].T
    pad_pos = np.vstack([np.zeros((1, EMB_POS), f32), emb_pos])    # [17, 4]
    pad_team = np.vstack([np.zeros((1, EMB_TEAM), f32), emb_team])  # [10, 3]
    WxT[OH_P0:OH_T0] = pad_pos @ Uall_w[:, NUM_CONT:NUM_CONT + EMB_POS].T
    WxT[OH_T0:OH_T0 + NCLS_TEAM + 1] = (
        pad_team @ Uall_w[:, NUM_CONT + EMB_POS:].T
    )

    # h-side weights with the full gate bias folded in as an extra row
    # (state tiles carry a constant-1 row at partition HID).
    WallT = np.concatenate([Wall_w.T, (Wall_b + Uall_b)[None, :]], 0)  # [101, 400]
    WdT = np.concatenate([Wd_w.T, Wd_b[None, :]], 0)                   # [101, 100]
    linT = np.concatenate([lin_w.T, lin_b[None, :]], 0)                # [101, 100]

    # Mean-aggregation matrix: M[s, d] = count(s->d) / max(deg(d), 1)
    src, dst = edge_index[0], edge_index[1]
    cnt = np.zeros((N, N), f32)
    np.add.at(cnt, (src, dst), 1.0)
    deg = np.maximum(cnt.sum(axis=0), 1.0)
    Mmat = cnt / deg[None, :]

    # iota rows for the merged one-hot compare: [0..16 | 0..9, -1 x7],
    # replicated for each timestep of an 8-step block
    iota2 = np.concatenate([
        np.arange(NCLS_POS + 1, dtype=f32),
        np.concatenate([np.arange(NCLS_TEAM + 1, dtype=f32),
                        -np.ones(17 - (NCLS_TEAM + 1), f32)]),
    ])
    iota2b = np.tile(iota2, 8)  # [8*34]

    import ml_dtypes
    bf = ml_dtypes.bfloat16
    return {
        "WxT": WxT.astype(bf),
        "WallT": WallT,
        "WdT": WdT,
        "linT": linT,
        "Mmat": np.ascontiguousarray(Mmat, f32),
        "s1l": np.ascontiguousarray(np.asarray(inputs["sage1_l"], f32).T),   # [100, 64]
        "s1r": np.ascontiguousarray(np.asarray(inputs["sage1_r"], f32).T),   # [100, 64]
        "s1b": np.ascontiguousarray(np.asarray(inputs["sage1_lb"], f32)[:, None]),  # [64, 1]
        "s2l": np.ascontiguousarray(np.asarray(inputs["sage2_l"], f32).T),   # [64, 32]
        "s2r": np.ascontiguousarray(np.asarray(inputs["sage2_r"], f32).T),   # [64, 32]
        "s2b": np.ascontiguousarray(np.asarray(inputs["sage2_lb"], f32)[:, None]),  # [32, 1]
        "ow": np.ascontiguousarray(np.asarray(inputs["out_w"], f32).T),      # [32, 2]
        "ob": np.ascontiguousarray(np.asarray(inputs["out_b"], f32)[:, None]),      # [2, 1]
        "iota2b": np.tile(iota2b, (120, 1)).astype(bf),                                # [120, 272]
        "hcinit": np.concatenate(
            [np.zeros((HID, R), f32), np.ones((1, R), f32)], 0
        ),  # [101, R]: zero state + constant-1 bias row
        "ident": np.eye(128, dtype=f32),
        "identb": np.eye(128, dtype=bf),
    }


# ---------------------------------------------------------------- device IR
def build_module(Wsteps=W):
    import concourse.bass as bass
    import concourse.tile as tile
    from concourse import bacc, mybir

    f32 = mybir.dt.float32
    f32r = mybir.dt.float32r
    bf16 = mybir.dt.bfloat16
    AF = mybir.ActivationFunctionType
    EQ = mybir.AluOpType.is_equal
    PSUM = bass.MemorySpace.PSUM

    def r(ap):
        # float32r view: same 4-byte data, single-pass matmul when N>=256
        return ap.bitcast(f32r)

    nc = bacc.Bacc(
        "TRN2", target_bir_lowering=False, debug=False, num_devices=NCORES
    )

    # All matmul-feeding tensors are float32r end-to-end (host arrays stay
    # np.float32; f32r is the same 4-byte encoding).
    X_in = nc.declare_dram_parameter("X", [BL, W, N, F_IN], bf16, isOutput=False)
    ts_in = nc.declare_dram_parameter("ts", [BL, W, N], f32r, isOutput=False)
    w_in = {}
    bf16_params = {"WxT", "iota2b", "identb"}
    for name, shape in [
        ("WxT", [XC, G4]), ("WallT", [HID + 1, G4]), ("WdT", [HID + 1, HID]),
        ("linT", [HID + 1, HID]), ("Mmat", [N, N]),
        ("s1l", [HID, 64]), ("s1r", [HID, 64]), ("s1b", [64, 1]),
        ("s2l", [64, 32]), ("s2r", [64, 32]), ("s2b", [32, 1]),
        ("ow", [32, 2]), ("ob", [2, 1]),
        ("iota2b", [120, 8 * 34]), ("hcinit", [HID + 1, R]),
        ("ident", [128, 128]), ("identb", [128, 128]),
    ]:
        w_in[name] = nc.declare_dram_parameter(
            name, shape, bf16 if name in bf16_params else f32r, isOutput=False
        )
    # device-natural layout [k, b, n]; host transposes to [b, n, k]
    out_ext = nc.declare_dram_parameter("out", [2, BL, N], f32, isOutput=True)

    with tile.TileContext(nc) as tc:
        with (
            tc.tile_pool(name="consts", bufs=1) as consts,
            tc.tile_pool(name="state", bufs=1) as state,
        ):
            # ---- load constants / weights
            wt = {}
            for name, ext in w_in.items():
                wt[name] = consts.tile(
                    list(ext.shape), ext.dtype, tag=name, name=name
                )
                nc.gpsimd.dma_start(out=wt[name][:], in_=ext[:])

            # ---- persistent state: h/c feature-major with const-1 bias row
            # (row HID stays 1.0 forever; per-step writes touch rows 0:HID)
            hT = state.tile([HID + 1, R], f32r, tag="hT")
            cT = state.tile([HID + 1, R], f32r, tag="cT")
            nc.gpsimd.dma_start(out=hT[:], in_=w_in["hcinit"][:])
            nc.gpsimd.dma_start(out=cT[:], in_=w_in["hcinit"][:])

            # ---- ts - 1, stored [64, 2, R]: row t at (partition t%64, block t//64)
            tsm1 = state.tile([64, 2, R], f32r, tag="tsm1")
            ts_jp = ts_in.rearrange("b (j p) n -> j p b n", p=64)
            for j in range(2):
                nc.sync.dma_start(
                    out=tsm1[:, j, :].rearrange("p (b n) -> p b n", n=N),
                    in_=ts_jp[j],
                )
            nc.vector.tensor_scalar_add(tsm1[:], tsm1[:], -1.0)
            # stage ts-1 to DRAM so per-step partition-broadcast DMAs can
            # read it with a flat 0-step AP (SBUF sources cannot broadcast)
            tsm1_d = nc.dram_tensor("tsm1_d", [64, 2, R], f32)
            nc.sync.dma_start(out=tsm1_d[:], in_=tsm1[:].bitcast(f32))

            nodesT = state.tile([HID, R], f32r, tag="nodesT")

            TB = 8  # timestep block for X/ts prefetch
            Xnb = X_in.rearrange("b t n f -> b n t f")

            with (
                tc.tile_pool(name="xs", bufs=2) as xs_pool,
                tc.tile_pool(name="xf", bufs=4) as xf_pool,
                tc.tile_pool(name="gsb", bufs=3) as gsb_pool,
                tc.tile_pool(name="tsb", bufs=3) as tsb_pool,
                tc.tile_pool(name="work", bufs=3) as work,
                tc.tile_pool(name="pg", bufs=1, space=PSUM) as pg_pool,
                tc.tile_pool(name="pxf", bufs=1, space=PSUM) as pxf_pool,
                tc.tile_pool(name="pd", bufs=1, space=PSUM) as pd_pool,
            ):
                xraw = [None] * 3
                TRIPLES = [(0, 3), (3, 3), (6, 2)]
                for t in range(Wsteps):
                    tl = t % TB
                    if tl == 0:
                        # per-b DMAs stack 3 graphs per tile: [120, TB, 100]
                        for k, (b0, nb) in enumerate(TRIPLES):
                            rows = N * nb
                            xt = xs_pool.tile([120, TB, XC], bf16,
                                              tag=f"xs{k}", name=f"xs{k}")
                            for i in range(nb):
                                nc.sync.dma_start(
                                    out=xt[N * i:N * (i + 1), :, 0:F_IN],
                                    in_=Xnb[b0 + i, :, t:t + TB, :],
                                )
                            # merged one-hot: both categorical cols, all TB
                            # steps, all stacked graphs in one op
                            nc.vector.tensor_tensor(
                                out=xt[:rows, :, OH_P0:XC].rearrange(
                                    "p t (g k) -> p t g k", k=17
                                ),
                                in0=wt["iota2b"][0:rows, :].rearrange(
                                    "p (t g k) -> p t g k", t=TB, k=17
                                ),
                                in1=xt[
                                    :rows, :, NUM_CONT:NUM_CONT + 2
                                ].to_broadcast([rows, TB, 2, 17]),
                                op=EQ,
                            )
                            xraw[k] = xt

                    # ------- per-step transposes -> xfT [100, 320]
                    pxf = pxf_pool.tile([XC, R], bf16, tag="pxf")
                    for k, (b0, nb) in enumerate(TRIPLES):
                        rows = N * nb
                        nc.tensor.transpose(
                            pxf[:, 120 * k:120 * k + rows],
                            xraw[k][:rows, tl, :],
                            wt["identb"][:rows, :rows],
                        )
                    xfT = xf_pool.tile([XC, R], bf16, tag="xfT")
                    nc.any.tensor_copy(out=xfT[:], in_=pxf[:])

                    # ------- ts-1 broadcast across partitions via SWDGE DMA
                    # (gpsimd is otherwise idle; src re-reads one partition)
                    tsb = tsb_pool.tile([HID, R], f32, tag="tsb")
                    ts_row = tsm1_d[t % 64, t // 64, :]
                    nc.gpsimd.dma_start(
                        out=tsb[:],
                        in_=bass.AP(
                            tensor=ts_row.tensor,
                            offset=ts_row.offset,
                            ap=[[0, HID], [1, R]],
                        ),
                    )

                    # ------- c path: c_adj = c + tanh(Wd@c + bd) * (ts-1)
                    pd = pd_pool.tile([HID, R], f32, tag="pd")
                    nc.tensor.matmul(pd, wt["WdT"][:], cT[:], start=True, stop=True)
                    cs1 = work.tile([HID, R], f32, tag="cs1")
                    nc.scalar.activation(cs1[:], pd, AF.Tanh)
                    t1 = work.tile([HID, R], f32, tag="t1")
                    nc.vector.tensor_mul(t1[:], cs1[:], tsb[:])
                    cadj = work.tile([HID, R], f32, tag="cadj")
                    nc.vector.tensor_add(cadj[:], cT[0:HID, :].bitcast(f32), t1[:])

                    # ------- gates: psum[g] = WxT_g.T @ xfT + WallT_g.T @ h1
                    # split across two psum tiles (pgA double-buffered) so
                    # next step's x-side matmuls can start before sigmoid
                    # consumes the previous gates
                    pgA = pg_pool.tile([HID, 2, 512], f32, tag="pgA", bufs=2)
                    pgB = pg_pool.tile([HID, 2, 512], f32, tag="pgB", bufs=1)
                    halves = (pgA, pgB)
                    # order: (f,i) x then h parts first so sigmoid A can
                    # start while (o,ct) matmuls still run
                    for g in (0, 1):
                        nc.tensor.matmul(
                            halves[0][:, g, 0:R],
                            wt["WxT"][:, HID * g:HID * (g + 1)],
                            xfT[:], start=True, stop=False,
                        )
                    for g in (0, 1):
                        nc.tensor.matmul(
                            halves[0][:, g, 0:R],
                            wt["WallT"][:, HID * g:HID * (g + 1)],
                            hT[:], start=False, stop=True,
                        )
                    for g in (2, 3):
                        nc.tensor.matmul(
                            halves[1][:, g - 2, 0:R],
                            wt["WxT"][:, HID * g:HID * (g + 1)],
                            xfT[:], start=True, stop=False,
                        )
                    for g in (2, 3):
                        nc.tensor.matmul(
                            halves[1][:, g - 2, 0:R],
                            wt["WallT"][:, HID * g:HID * (g + 1)],
                            hT[:], start=False, stop=True,
                        )
                    gs = gsb_pool.tile([HID, 4, R], f32, tag="gs")
                    nc.scalar.activation(gs[:, 0:2, :], pgA[:, :, 0:R], AF.Sigmoid)
                    nc.scalar.activation(gs[:, 2:4, :], pgB[:, :, 0:R], AF.Sigmoid)

                    # ------- state update: c = f*c_adj + i*ct ; h = o*tanh(c)
                    t2 = work.tile([HID, R], f32, tag="t2")
                    nc.vector.tensor_mul(t2[:], gs[:, 0, :], cadj[:])
                    t3 = work.tile([HID, R], f32, tag="t3")
                    nc.vector.tensor_mul(t3[:], gs[:, 1, :], gs[:, 3, :])
                    nc.vector.tensor_add(cT[0:HID, :], t2[:], t3[:])
                    tnc = work.tile([HID, R], f32, tag="tnc")
                    nc.scalar.activation(tnc[:], cT[0:HID, :].bitcast(f32), AF.Tanh)
                    nc.vector.tensor_mul(hT[0:HID, :], gs[:, 2, :], tnc[:])

                # ---- output linear: nodes = relu(lin @ h + lb)
                pl = pd_pool.tile([HID, R], f32, tag="pd")
                nc.tensor.matmul(pl, wt["linT"][:], hT[:], start=True, stop=True)
                nc.scalar.activation(nodesT[:], pl, AF.Relu)

            # ---------------- GCN: two SAGE layers + output proj
            with (
                tc.tile_pool(name="gc", bufs=2) as gc,
                tc.tile_pool(name="gcs", bufs=1) as gcs,
                tc.tile_pool(name="gp", bufs=2, space=PSUM) as gp,
                tc.tile_pool(name="gp1", bufs=1, space=PSUM) as gp1,
            ):
                def mean_agg(srcT, hid):
                    """srcT: [hid, R] feature-major -> aggT [hid, R]."""
                    aggT = gcs.tile([hid, R], f32r, tag=f"agg{hid}", name="aggT")
                    for b in range(BL):
                        cols = srcT[:, N * b:N * (b + 1)]   # [hid, 40] graph b
                        ptr = gp.tile([N, 128], f32, tag="ptr")
                        nc.tensor.transpose(
                            r(ptr[:, 0:hid]), cols, wt["ident"][:hid, :hid]
                        )
                        nbm = gc.tile([N, 128], f32r, tag="nbm")
                        nc.any.tensor_copy(out=nbm[:, 0:hid], in_=ptr[:, 0:hid])
                        pa = gp.tile([128, N], f32, tag="pa")
                        nc.tensor.matmul(
                            pa[0:hid, :], nbm[:, 0:hid], wt["Mmat"][:],
                            start=True, stop=True,
                        )
                        nc.any.tensor_copy(
                            out=aggT[:, N * b:N * (b + 1)], in_=pa[0:hid, :]
                        )
                    return aggT

                agg1 = mean_agg(nodesT, HID)
                pg1 = gp1.tile([64, R], f32, tag="pg1")
                nc.tensor.matmul(pg1, wt["s1l"][:], agg1[:], start=True, stop=False)
                nc.tensor.matmul(pg1, wt["s1r"][:], nodesT[:], start=False, stop=True)
                g1T = gcs.tile([64, R], f32r, tag="g1T")
                nc.scalar.activation(g1T[:], pg1, AF.Relu, bias=wt["s1b"][:].bitcast(f32))

                agg2 = mean_agg(g1T, 64)
                pg2 = gp1.tile([32, R], f32, tag="pg2")
                nc.tensor.matmul(pg2, wt["s2l"][:], agg2[:], start=True, stop=False)
                nc.tensor.matmul(pg2, wt["s2r"][:], g1T[:], start=False, stop=True)
                g2T = gcs.tile([32, R], f32r, tag="g2T")
                nc.scalar.activation(g2T[:], pg2, AF.Relu, bias=wt["s2b"][:].bitcast(f32))

                po = gp1.tile([2, R], f32, tag="po")
                nc.tensor.matmul(po, wt["ow"][:], g2T[:], start=True, stop=True)
                oT = gcs.tile([2, R], f32, tag="oT")
                nc.scalar.activation(oT[:], po, AF.Relu, bias=wt["ob"][:].bitcast(f32))

                nc.sync.dma_start(
                    out=out_ext.rearrange("k b n -> k (b n)"), in_=oT[:]
                )

    nc.compile()
    return nc


# ---------------------------------------------------------------- execution
_CACHE = {}


def _get_module():
    if "nc" not in _CACHE:
        _CACHE["nc"] = build_module()
    return _CACHE["nc"]


def make_in_maps(inputs):
    f32 = np.float32
    import ml_dtypes
    X = np.ascontiguousarray(np.asarray(inputs["X"], f32).astype(ml_dtypes.bfloat16))
    ts = np.ascontiguousarray(np.asarray(inputs["ts_list"], f32))
    wts = _host_weights(inputs)
    in_maps = []
    for c in range(NCORES):
        m = {"X": X[c * BL:(c + 1) * BL], "ts": ts[c * BL:(c + 1) * BL]}
        m.update(wts)
        in_maps.append(m)
    return in_maps


def kernel(**inputs) -> np.ndarray:
    from concourse.bass_utils import run_bass_kernel_spmd

    nc = _get_module()
    in_maps = make_in_maps(inputs)
    res = run_bass_kernel_spmd(nc, in_maps, list(range(NCORES)))
    outs = [
        np.transpose(res.results[c]["out"], (1, 2, 0)) for c in range(NCORES)
    ]
    return np.ascontiguousarray(np.concatenate(outs, axis=0).astype(np.float32))



# revision 5
# speedup vs baseline: 1.1794x; 1.1794x over previous
"""AgentImputer Trainium2 kernel.

Contract: kernel(**inputs) takes the FULL unsharded inputs (as produced by
reference.setup_inputs()) and returns the FULL output [64, 40, 2] float32.

Strategy: data-parallel over batch B=64 across 8 NeuronCores (8 batches /
core -> 320 folded LSTM rows per core). The 128-step TimeLSTM runs
feature-major ([hid, row] tiles); categorical embeddings fold into the
input matmul via one-hot rows; biases fold into matmuls via a constant-1
state row. The recurrent loop is software-pipelined as TWO independent
column groups (rows 0:160 / 160:320) so the serial h->gates->c->h chain of
one group overlaps engine work of the other. All elementwise state math is
bf16 (DVE 2x packed mode); sigmoid over all 4 gates of a group is a single
ACT instruction; (ts-1) is host-precomputed and DMA-broadcast per step; the
per-graph GCN (shared edge_index) is dense [40,40] mean-aggregation matmuls.
"""

import sys

import numpy as np

sys.path.insert(0, "/opt/trn_rl_repo")

# ---------------------------------------------------------------- constants
B, W, N, F_IN = 64, 128, 40, 66
HID = 100
NUM_CONT = 64
NCLS_POS, NCLS_TEAM = 16, 9
EMB_POS, EMB_TEAM = 4, 3
NCORES = 8
BL = B // NCORES          # 8 local batch elems per core
R = BL * N                # 320 rows per core; row j = 40*b_local + n
RG = R // 2               # columns per pipeline group
OH_P0 = 66                # one-hot pos cols [66:83)
OH_T0 = 83                # one-hot team cols [83:100) (entries 10..16 pad)
XC = 100                  # xs tile feature columns
G4 = 4 * HID
TB = 8                    # timestep block for X prefetch


# ---------------------------------------------------------------- host prep
def _host_weights(inputs):
    import ml_dtypes
    bf = ml_dtypes.bfloat16
    f32 = np.float32
    Uall_w = np.asarray(inputs["Uall_w"], f32)       # [400, 71]
    Uall_b = np.asarray(inputs["Uall_b"], f32)       # [400]
    Wall_w = np.asarray(inputs["Wall_w"], f32)       # [400, 100]
    Wall_b = np.asarray(inputs["Wall_b"], f32)       # [400]
    Wd_w = np.asarray(inputs["Wd_w"], f32)           # [100, 100]
    Wd_b = np.asarray(inputs["Wd_b"], f32)           # [100]
    lin_w = np.asarray(inputs["lin_w"], f32)         # [100, 100]
    lin_b = np.asarray(inputs["lin_b"], f32)         # [100]
    emb_pos = np.asarray(inputs["emb_pos"], f32)     # [16, 4]
    emb_team = np.asarray(inputs["emb_team"], f32)   # [9, 3]
    edge_index = np.asarray(inputs["edge_index"]).astype(np.int64)  # [2, E]

    # Input-side weights [100, 400]: rows 0:64 continuous features; rows
    # 64,65 (raw categorical codes riding along in the transposed tile) get
    # zero weights; rows 66:83 / 83:93 are one-hot rows with the embedding
    # tables pre-multiplied in (code 0 == missing -> zero row); 93:100 pad.
    WxT = np.zeros((XC, G4), f32)
    WxT[0:NUM_CONT] = Uall_w[:, 0:NUM_CONT].T
    pad_pos = np.vstack([np.zeros((1, EMB_POS), f32), emb_pos])    # [17, 4]
    pad_team = np.vstack([np.zeros((1, EMB_TEAM), f32), emb_team])  # [10, 3]
    WxT[OH_P0:OH_T0] = pad_pos @ Uall_w[:, NUM_CONT:NUM_CONT + EMB_POS].T
    WxT[OH_T0:OH_T0 + NCLS_TEAM + 1] = (
        pad_team @ Uall_w[:, NUM_CONT + EMB_POS:].T
    )

    # h-side weights with the full gate bias folded in as an extra row
    # (state tiles carry a constant-1 row at partition HID).
    WallT = np.concatenate([Wall_w.T, (Wall_b + Uall_b)[None, :]], 0)  # [101, 400]
    WdT = np.concatenate([Wd_w.T, Wd_b[None, :]], 0)                   # [101, 100]
    linT = np.concatenate([lin_w.T, lin_b[None, :]], 0)                # [101, 100]

    # Mean-aggregation matrix: M[s, d] = count(s->d) / max(deg(d), 1)
    src, dst = edge_index[0], edge_index[1]
    cnt = np.zeros((N, N), f32)
    np.add.at(cnt, (src, dst), 1.0)
    deg = np.maximum(cnt.sum(axis=0), 1.0)
    Mmat = cnt / deg[None, :]

    # iota rows for the merged one-hot compare: [0..16 | 0..9, -1 x7],
    # replicated for each timestep of an 8-step block
    iota2 = np.concatenate([
        np.arange(NCLS_POS + 1, dtype=f32),
        np.concatenate([np.arange(NCLS_TEAM + 1, dtype=f32),
                        -np.ones(17 - (NCLS_TEAM + 1), f32)]),
    ])
    iota2b = np.tile(iota2, TB)  # [TB*34]

    return {
        "WxT": WxT.astype(bf),
        "WallT": WallT.astype(bf),
        "WdT": WdT.astype(bf),
        "linT": linT.astype(bf),
        "Mmat": np.ascontiguousarray(Mmat, f32),
        "s1l": np.ascontiguousarray(np.asarray(inputs["sage1_l"], f32).T),   # [100, 64]
        "s1r": np.ascontiguousarray(np.asarray(inputs["sage1_r"], f32).T),   # [100, 64]
        "s1b": np.ascontiguousarray(np.asarray(inputs["sage1_lb"], f32)[:, None]),  # [64, 1]
        "s2l": np.ascontiguousarray(np.asarray(inputs["sage2_l"], f32).T),   # [64, 32]
        "s2r": np.ascontiguousarray(np.asarray(inputs["sage2_r"], f32).T),   # [64, 32]
        "s2b": np.ascontiguousarray(np.asarray(inputs["sage2_lb"], f32)[:, None]),  # [32, 1]
        "ow": np.ascontiguousarray(np.asarray(inputs["out_w"], f32).T),      # [32, 2]
        "ob": np.ascontiguousarray(np.asarray(inputs["out_b"], f32)[:, None]),      # [2, 1]
        "iota2b": np.tile(iota2b, (120, 1)).astype(bf),                      # [120, TB*34]
        "hcinit": np.concatenate(
            [np.zeros((HID, R), f32), np.ones((1, R), f32)], 0
        ).astype(bf),  # [101, R]: zero state + constant-1 bias row
        "ident": np.eye(128, dtype=f32),
        "identb": np.eye(128, dtype=bf),
    }


# ---------------------------------------------------------------- device IR
def build_module(Wsteps=W):
    import concourse.bass as bass
    import concourse.tile as tile
    from concourse import bacc, mybir

    f32 = mybir.dt.float32
    f32r = mybir.dt.float32r
    bf16 = mybir.dt.bfloat16
    AF = mybir.ActivationFunctionType
    EQ = mybir.AluOpType.is_equal
    ADD = mybir.AluOpType.add
    PSUM = bass.MemorySpace.PSUM

    def r(ap):
        return ap.bitcast(f32r)

    nc = bacc.Bacc(
        "TRN2", target_bir_lowering=False, debug=False, num_devices=NCORES
    )

    X_in = nc.declare_dram_parameter("X", [BL, W, N, F_IN], bf16, isOutput=False)
    # host-precomputed (ts-1), replicated across 100 partitions: [100, W, R]
    tsm1_in = nc.declare_dram_parameter("tsm1", [HID, W, R], bf16, isOutput=False)
    w_in = {}
    bf16_params = {"WxT", "WallT", "WdT", "linT", "iota2b", "identb", "hcinit"}
    for name, shape in [
        ("WxT", [XC, G4]), ("WallT", [HID + 1, G4]), ("WdT", [HID + 1, HID]),
        ("linT", [HID + 1, HID]), ("Mmat", [N, N]),
        ("s1l", [HID, 64]), ("s1r", [HID, 64]), ("s1b", [64, 1]),
        ("s2l", [64, 32]), ("s2r", [64, 32]), ("s2b", [32, 1]),
        ("ow", [32, 2]), ("ob", [2, 1]),
        ("iota2b", [120, TB * 34]), ("hcinit", [HID + 1, R]),
        ("ident", [128, 128]), ("identb", [128, 128]),
    ]:
        w_in[name] = nc.declare_dram_parameter(
            name, shape, bf16 if name in bf16_params else f32r, isOutput=False
        )
    # device-natural layout [k, b, n]; host transposes to [b, n, k]
    out_ext = nc.declare_dram_parameter("out", [2, BL, N], f32, isOutput=True)

    GSL = [slice(0, RG), slice(RG, R)]

    with tile.TileContext(nc) as tc:
        with (
            tc.tile_pool(name="consts", bufs=1) as consts,
            tc.tile_pool(name="state", bufs=1) as state,
        ):
            # ---- load constants / weights
            wt = {}
            for name, ext in w_in.items():
                wt[name] = consts.tile(
                    list(ext.shape), ext.dtype, tag=name, name=name
                )
                nc.gpsimd.dma_start(out=wt[name][:], in_=ext[:])

            # ---- persistent state: h/c feature-major with const-1 bias row
            hT = state.tile([HID + 1, R], bf16, tag="hT")
            cT = state.tile([HID + 1, R], bf16, tag="cT")
            nc.gpsimd.dma_start(out=hT[:], in_=w_in["hcinit"][:])
            nc.gpsimd.dma_start(out=cT[:], in_=w_in["hcinit"][:])

            nodesT = state.tile([HID, R], f32r, tag="nodesT")

            Xnb = X_in.rearrange("b t n f -> b n t f")

            with (
                tc.tile_pool(name="xs", bufs=2) as xs_pool,
                tc.tile_pool(name="xf", bufs=2) as xf_pool,
                tc.tile_pool(name="tsb", bufs=2) as tsb_pool,
                tc.tile_pool(name="sg", bufs=2) as sg_pool,
                tc.tile_pool(name="work", bufs=2) as work,
                tc.tile_pool(name="pga", bufs=1, space=PSUM) as pga_pool,
                tc.tile_pool(name="pgb", bufs=1, space=PSUM) as pgb_pool,
                tc.tile_pool(name="pd", bufs=1, space=PSUM) as pd_pool,
                tc.tile_pool(name="pxf", bufs=2, space=PSUM) as pxf_pool,
            ):
                xraw = [None] * 3
                TRIPLES = [(0, 3), (3, 3), (6, 2)]
                for t in range(Wsteps):
                    tl = t % TB
                    if tl == 0:
                        # per-b DMAs stack 3 graphs per tile: [120, TB, 100]
                        for k, (b0, nb) in enumerate(TRIPLES):
                            rows = N * nb
                            xt = xs_pool.tile([120, TB, XC], bf16,
                                              tag=f"xs{k}", name=f"xs{k}")
                            for i in range(nb):
                                nc.sync.dma_start(
                                    out=xt[N * i:N * (i + 1), :, 0:F_IN],
                                    in_=Xnb[b0 + i, :, t:t + TB, :],
                                )
                            # merged one-hot: both categorical cols, all TB
                            # steps, all stacked graphs in one op
                            nc.vector.tensor_tensor(
                                out=xt[:rows, :, OH_P0:XC].rearrange(
                                    "p t (g k) -> p t g k", k=17
                                ),
                                in0=wt["iota2b"][0:rows, :].rearrange(
                                    "p (t g k) -> p t g k", t=TB, k=17
                                ),
                                in1=xt[
                                    :rows, :, NUM_CONT:NUM_CONT + 2
                                ].to_broadcast([rows, TB, 2, 17]),
                                op=EQ,
                            )
                            xraw[k] = xt

                    # ------- per-step transposes -> xfT [100, 320] bf16
                    pxf = pxf_pool.tile([XC, R], bf16, tag="pxf")
                    for k, (b0, nb) in enumerate(TRIPLES):
                        rows = N * nb
                        nc.tensor.transpose(
                            pxf[:, 120 * k:120 * k + rows],
                            xraw[k][:rows, tl, :],
                            wt["identb"][:rows, :rows],
                        )
                    xfT = xf_pool.tile([XC, R], bf16, tag="xfT")
                    nc.vector.tensor_scalar_add(xfT[:], pxf[:], 0.0)

                    # ------- (ts-1) broadcast tile: plain HWDGE DMA from the
                    # host-replicated [100, W, R] tensor (no engine compute)
                    tsb = tsb_pool.tile([HID, R], bf16, tag="tsb")
                    nc.sync.dma_start(out=tsb[:], in_=tsm1_in[:, t, :])

                    # ------- c path (full width, off the critical h-chain):
                    # cs1 = tanh(Wd@c + bd)
                    pd = pd_pool.tile([HID, 512], f32, tag="pd")
                    nc.tensor.matmul(pd[:, 0:R], wt["WdT"][:], cT[:],
                                     start=True, stop=True)
                    cs1 = work.tile([HID, R], bf16, tag="cs1")
                    nc.scalar.activation(cs1[:], pd[:, 0:R], AF.Tanh)

                    t1 = work.tile([HID, R], bf16, tag="t1")
                    cadj = work.tile([HID, R], bf16, tag="cadj")
                    t2 = work.tile([HID, R], bf16, tag="t2")
                    t3 = work.tile([HID, R], bf16, tag="t3")
                    tnc = work.tile([HID, R], bf16, tag="tnc")
                    sg = sg_pool.tile([HID, 4, R], bf16, tag="sg")

                    pgA = pga_pool.tile([HID, 2, 512], f32, tag="pgA", name="pgA")
                    pgB = pgb_pool.tile([HID, 2, 512], f32, tag="pgB", name="pgB")
                    pgrp = [pgA, pgB]

                    for gi in range(2):
                        gsl = GSL[gi]
                        pg = pgrp[gi]

                        # t1 = cs1*(ts-1); cadj = c + t1 (cadj on gpsimd)
                        nc.vector.tensor_mul(t1[:, gsl], cs1[:, gsl], tsb[:, gsl])
                        nc.gpsimd.tensor_tensor(
                            out=cadj[:, gsl], in0=cT[0:HID, gsl],
                            in1=t1[:, gsl], op=ADD,
                        )

                        # gates: psum[g] = WxT_g.T @ x + WallT_g.T @ h
                        # gate g lives at pg[:, g//2, (g%2)*RG : +RG].
                        # start=True arms the whole 2KB bank as pending-zero,
                        # so only the FIRST matmul per bank sets it; the
                        # bank's group closes with stop=True on the last.
                        for g in range(4):
                            sl = slice((g % 2) * RG, (g % 2) * RG + RG)
                            nc.tensor.matmul(
                                pg[:, g // 2, sl],
                                wt["WxT"][:, HID * g:HID * (g + 1)],
                                xfT[:, gsl], start=(g % 2 == 0), stop=False,
                            )
                        for g in range(4):
                            sl = slice((g % 2) * RG, (g % 2) * RG + RG)
                            nc.tensor.matmul(
                                pg[:, g // 2, sl],
                                wt["WallT"][:, HID * g:HID * (g + 1)],
                                hT[:, gsl], start=False, stop=(g % 2 == 1),
                            )

                        # one sigmoid instruction for all 4 gates of group
                        nc.scalar.activation(
                            sg[:, :, gsl].rearrange("p (b s) c -> p b s c", s=2),
                            pg[:, :, 0:2 * RG].rearrange(
                                "p b (s c) -> p b s c", c=RG
                            ),
                            AF.Sigmoid,
                        )

                        # state update: c' = f*cadj + i*ct ; h' = o*tanh(c')
                        # gate order along dim1 of sg: f, o | i, ct
                        # (g=0 -> [:,0,0:RG]=f, g=1 -> [:,0,RG:]=i,
                        #  g=2 -> [:,1,0:RG]=o, g=3 -> [:,1,RG:]=ct)
                        f_g = sg[:, 0, gsl]
                        i_g = sg[:, 1, gsl]
                        o_g = sg[:, 2, gsl]
                        ct_g = sg[:, 3, gsl]
                        nc.vector.tensor_mul(t3[:, gsl], i_g, ct_g)
                        nc.vector.tensor_mul(t2[:, gsl], f_g, cadj[:, gsl])
                        nc.vector.tensor_add(cT[0:HID, gsl], t2[:, gsl], t3[:, gsl])
                        nc.scalar.activation(tnc[:, gsl], cT[0:HID, gsl], AF.Tanh)
                        nc.vector.tensor_mul(hT[0:HID, gsl], o_g, tnc[:, gsl])

                # ---- output linear: nodes = relu(lin @ h + lb)
                pl = pd_pool.tile([HID, 512], f32, tag="pd")
                nc.tensor.matmul(pl[:, 0:R], wt["linT"][:], hT[:],
                                 start=True, stop=True)
                nc.scalar.activation(nodesT[:], pl[:, 0:R], AF.Relu)

            # ---------------- GCN: two SAGE layers + output proj
            with (
                tc.tile_pool(name="gc", bufs=2) as gc,
                tc.tile_pool(name="gcs", bufs=1) as gcs,
                tc.tile_pool(name="gp", bufs=2, space=PSUM) as gp,
                tc.tile_pool(name="gp1", bufs=1, space=PSUM) as gp1,
            ):
                def mean_agg(srcT, hid):
                    """srcT: [hid, R] feature-major -> aggT [hid, R]."""
                    aggT = gcs.tile([hid, R], f32r, tag=f"agg{hid}", name="aggT")
                    for b in range(BL):
                        cols = srcT[:, N * b:N * (b + 1)]   # [hid, 40] graph b
                        ptr = gp.tile([N, 128], f32, tag="ptr")
                        nc.tensor.transpose(
                            r(ptr[:, 0:hid]), cols, wt["ident"][:hid, :hid]
                        )
                        nbm = gc.tile([N, 128], f32r, tag="nbm")
                        nc.any.tensor_copy(out=nbm[:, 0:hid], in_=ptr[:, 0:hid])
                        pa = gp.tile([128, N], f32, tag="pa")
                        nc.tensor.matmul(
                            pa[0:hid, :], nbm[:, 0:hid], wt["Mmat"][:],
                            start=True, stop=True,
                        )
                        nc.any.tensor_copy(
                            out=aggT[:, N * b:N * (b + 1)], in_=pa[0:hid, :]
                        )
                    return aggT

                agg1 = mean_agg(nodesT, HID)
                pg1 = gp1.tile([64, R], f32, tag="pg1")
                nc.tensor.matmul(pg1, wt["s1l"][:], agg1[:], start=True, stop=False)
                nc.tensor.matmul(pg1, wt["s1r"][:], nodesT[:], start=False, stop=True)
                g1T = gcs.tile([64, R], f32r, tag="g1T")
                nc.scalar.activation(g1T[:], pg1, AF.Relu, bias=wt["s1b"][:].bitcast(f32))

                agg2 = mean_agg(g1T, 64)
                pg2 = gp1.tile([32, R], f32, tag="pg2")
                nc.tensor.matmul(pg2, wt["s2l"][:], agg2[:], start=True, stop=False)
                nc.tensor.matmul(pg2, wt["s2r"][:], g1T[:], start=False, stop=True)
                g2T = gcs.tile([32, R], f32r, tag="g2T")
                nc.scalar.activation(g2T[:], pg2, AF.Relu, bias=wt["s2b"][:].bitcast(f32))

                po = gp1.tile([2, R], f32, tag="po")
                nc.tensor.matmul(po, wt["ow"][:], g2T[:], start=True, stop=True)
                oT = gcs.tile([2, R], f32, tag="oT")
                nc.scalar.activation(oT[:], po, AF.Relu, bias=wt["ob"][:].bitcast(f32))

                nc.sync.dma_start(
                    out=out_ext.rearrange("k b n -> k (b n)"), in_=oT[:]
                )

    nc.compile()
    return nc


# ---------------------------------------------------------------- execution
_CACHE = {}


def _get_module():
    if "nc" not in _CACHE:
        _CACHE["nc"] = build_module()
    return _CACHE["nc"]


def make_in_maps(inputs):
    f32 = np.float32
    import ml_dtypes
    bf = ml_dtypes.bfloat16
    X = np.ascontiguousarray(np.asarray(inputs["X"], f32).astype(bf))
    ts = np.asarray(inputs["ts_list"], f32)
    wts = _host_weights(inputs)
    in_maps = []
    for c in range(NCORES):
        tsl = ts[c * BL:(c + 1) * BL]                       # [BL, W, N]
        tsm1 = (tsl.transpose(1, 0, 2).reshape(W, R) - 1.0).astype(bf)
        tsm1_rep = np.ascontiguousarray(
            np.broadcast_to(tsm1[None], (HID, W, R))
        )
        m = {"X": X[c * BL:(c + 1) * BL], "tsm1": tsm1_rep}
        m.update(wts)
        in_maps.append(m)
    return in_maps


def kernel(**inputs) -> np.ndarray:
    from concourse.bass_utils import run_bass_kernel_spmd

    nc = _get_module()
    in_maps = make_in_maps(inputs)
    res = run_bass_kernel_spmd(nc, in_maps, list(range(NCORES)))
    outs = [
        np.transpose(res.results[c]["out"], (1, 2, 0)) for c in range(NCORES)
    ]
    return np.ascontiguousarray(np.concatenate(outs, axis=0).astype(np.float32))


# revision 7
# speedup vs baseline: 1.1988x; 1.0165x over previous
"""AgentImputer Trainium2 kernel.

Contract: kernel(**inputs) takes the FULL unsharded inputs (as produced by
reference.setup_inputs()) and returns the FULL output [64, 40, 2] float32.

Strategy: data-parallel over batch B=64 across 8 NeuronCores (8 batches /
core -> 320 folded LSTM rows per core). The 128-step TimeLSTM runs
feature-major ([hid, row] tiles); categorical embeddings fold into the
input matmul via one-hot rows; biases fold into matmuls via a constant-1
state row. The recurrent loop is software-pipelined as TWO independent
column groups (rows 0:160 / 160:320) so the serial h->gates->c->h chain of
one group overlaps engine work of the other. All elementwise state math is
bf16 (DVE 2x packed mode); sigmoid over all 4 gates of a group is a single
ACT instruction; (ts-1) is host-precomputed and DMA-broadcast per step; the
per-graph GCN (shared edge_index) is dense [40,40] mean-aggregation matmuls.
"""

import sys

import numpy as np

sys.path.insert(0, "/opt/trn_rl_repo")

# ---------------------------------------------------------------- constants
B, W, N, F_IN = 64, 128, 40, 66
HID = 100
NUM_CONT = 64
NCLS_POS, NCLS_TEAM = 16, 9
EMB_POS, EMB_TEAM = 4, 3
NCORES = 8
BL = B // NCORES          # 8 local batch elems per core
R = BL * N                # 320 rows per core; row j = 40*b_local + n
RG = R // 2               # columns per pipeline group
OH_P0 = 66                # one-hot pos cols [66:83)
OH_T0 = 83                # one-hot team cols [83:100) (entries 10..16 pad)
XC = 100                  # xs tile feature columns
G4 = 4 * HID
TB = 8                    # timestep block for X prefetch


# ---------------------------------------------------------------- host prep
def _host_weights(inputs):
    import ml_dtypes
    bf = ml_dtypes.bfloat16
    f32 = np.float32
    Uall_w = np.asarray(inputs["Uall_w"], f32)       # [400, 71]
    Uall_b = np.asarray(inputs["Uall_b"], f32)       # [400]
    Wall_w = np.asarray(inputs["Wall_w"], f32)       # [400, 100]
    Wall_b = np.asarray(inputs["Wall_b"], f32)       # [400]
    Wd_w = np.asarray(inputs["Wd_w"], f32)           # [100, 100]
    Wd_b = np.asarray(inputs["Wd_b"], f32)           # [100]
    lin_w = np.asarray(inputs["lin_w"], f32)         # [100, 100]
    lin_b = np.asarray(inputs["lin_b"], f32)         # [100]
    emb_pos = np.asarray(inputs["emb_pos"], f32)     # [16, 4]
    emb_team = np.asarray(inputs["emb_team"], f32)   # [9, 3]
    edge_index = np.asarray(inputs["edge_index"]).astype(np.int64)  # [2, E]

    # Input-side weights [100, 400]: rows 0:64 continuous features; rows
    # 64,65 (raw categorical codes riding along in the transposed tile) get
    # zero weights; rows 66:83 / 83:93 are one-hot rows with the embedding
    # tables pre-multiplied in (code 0 == missing -> zero row); 93:100 pad.
    WxT = np.zeros((XC, G4), f32)
    WxT[0:NUM_CONT] = Uall_w[:, 0:NUM_CONT].T
    pad_pos = np.vstack([np.zeros((1, EMB_POS), f32), emb_pos])    # [17, 4]
    pad_team = np.vstack([np.zeros((1, EMB_TEAM), f32), emb_team])  # [10, 3]
    WxT[OH_P0:OH_T0] = pad_pos @ Uall_w[:, NUM_CONT:NUM_CONT + EMB_POS].T
    WxT[OH_T0:OH_T0 + NCLS_TEAM + 1] = (
        pad_team @ Uall_w[:, NUM_CONT + EMB_POS:].T
    )

    # h-side weights with the full gate bias folded in as an extra row
    # (state tiles carry a constant-1 row at partition HID).
    WallT = np.concatenate([Wall_w.T, (Wall_b + Uall_b)[None, :]], 0)  # [101, 400]
    WdT = np.concatenate([Wd_w.T, Wd_b[None, :]], 0)                   # [101, 100]
    linT = np.concatenate([lin_w.T, lin_b[None, :]], 0)                # [101, 100]

    # Mean-aggregation matrix: M[s, d] = count(s->d) / max(deg(d), 1)
    src, dst = edge_index[0], edge_index[1]
    cnt = np.zeros((N, N), f32)
    np.add.at(cnt, (src, dst), 1.0)
    deg = np.maximum(cnt.sum(axis=0), 1.0)
    Mmat = cnt / deg[None, :]

    # iota rows for the merged one-hot compare: [0..16 | 0..9, -1 x7],
    # replicated for each timestep of an 8-step block
    iota2 = np.concatenate([
        np.arange(NCLS_POS + 1, dtype=f32),
        np.concatenate([np.arange(NCLS_TEAM + 1, dtype=f32),
                        -np.ones(17 - (NCLS_TEAM + 1), f32)]),
    ])
    iota2b = np.tile(iota2, TB)  # [TB*34]

    return {
        "WxT": WxT.astype(bf),
        "WallT": WallT.astype(bf),
        "WdT": WdT.astype(bf),
        "linT": linT.astype(bf),
        "Mmat": np.ascontiguousarray(Mmat, f32),
        "s1l": np.ascontiguousarray(np.asarray(inputs["sage1_l"], f32).T),   # [100, 64]
        "s1r": np.ascontiguousarray(np.asarray(inputs["sage1_r"], f32).T),   # [100, 64]
        "s1b": np.ascontiguousarray(np.asarray(inputs["sage1_lb"], f32)[:, None]),  # [64, 1]
        "s2l": np.ascontiguousarray(np.asarray(inputs["sage2_l"], f32).T),   # [64, 32]
        "s2r": np.ascontiguousarray(np.asarray(inputs["sage2_r"], f32).T),   # [64, 32]
        "s2b": np.ascontiguousarray(np.asarray(inputs["sage2_lb"], f32)[:, None]),  # [32, 1]
        "ow": np.ascontiguousarray(np.asarray(inputs["out_w"], f32).T),      # [32, 2]
        "ob": np.ascontiguousarray(np.asarray(inputs["out_b"], f32)[:, None]),      # [2, 1]
        "iota2b": np.tile(iota2b, (120, 1)).astype(bf),                      # [120, TB*34]
        "hcinit": np.concatenate(
            [np.zeros((HID, R), f32), np.ones((1, R), f32)], 0
        ).astype(bf),  # [101, R]: zero state + constant-1 bias row
        "ident": np.eye(128, dtype=f32),
        "identb": np.eye(128, dtype=bf),
    }


# ---------------------------------------------------------------- device IR
def build_module(Wsteps=W):
    import concourse.bass as bass
    import concourse.tile as tile
    from concourse import bacc, mybir

    f32 = mybir.dt.float32
    f32r = mybir.dt.float32r
    bf16 = mybir.dt.bfloat16
    AF = mybir.ActivationFunctionType
    EQ = mybir.AluOpType.is_equal
    ADD = mybir.AluOpType.add
    PSUM = bass.MemorySpace.PSUM

    def r(ap):
        return ap.bitcast(f32r)

    nc = bacc.Bacc(
        "TRN2", target_bir_lowering=False, debug=False, num_devices=NCORES
    )

    X_in = nc.declare_dram_parameter("X", [BL, W, N, F_IN], bf16, isOutput=False)
    # host-precomputed (ts-1), replicated across 100 partitions: [100, W, R]
    tsm1_in = nc.declare_dram_parameter("tsm1", [HID, W, R], bf16, isOutput=False)
    w_in = {}
    bf16_params = {"WxT", "WallT", "WdT", "linT", "iota2b", "identb", "hcinit"}
    for name, shape in [
        ("WxT", [XC, G4]), ("WallT", [HID + 1, G4]), ("WdT", [HID + 1, HID]),
        ("linT", [HID + 1, HID]), ("Mmat", [N, N]),
        ("s1l", [HID, 64]), ("s1r", [HID, 64]), ("s1b", [64, 1]),
        ("s2l", [64, 32]), ("s2r", [64, 32]), ("s2b", [32, 1]),
        ("ow", [32, 2]), ("ob", [2, 1]),
        ("iota2b", [120, TB * 34]), ("hcinit", [HID + 1, R]),
        ("ident", [128, 128]), ("identb", [128, 128]),
    ]:
        w_in[name] = nc.declare_dram_parameter(
            name, shape, bf16 if name in bf16_params else f32r, isOutput=False
        )
    # device-natural layout [k, b, n]; host transposes to [b, n, k]
    out_ext = nc.declare_dram_parameter("out", [2, BL, N], f32, isOutput=True)

    GSL = [slice(0, RG), slice(RG, R)]

    with tile.TileContext(nc) as tc:
        with (
            tc.tile_pool(name="consts", bufs=1) as consts,
            tc.tile_pool(name="state", bufs=1) as state,
        ):
            # ---- load constants / weights
            wt = {}
            for name, ext in w_in.items():
                wt[name] = consts.tile(
                    list(ext.shape), ext.dtype, tag=name, name=name
                )
                nc.gpsimd.dma_start(out=wt[name][:], in_=ext[:])

            # ---- persistent state: h/c feature-major with const-1 bias row
            hT = state.tile([HID + 1, R], bf16, tag="hT")
            cT = state.tile([HID + 1, R], bf16, tag="cT")
            nc.gpsimd.dma_start(out=hT[:], in_=w_in["hcinit"][:])
            nc.gpsimd.dma_start(out=cT[:], in_=w_in["hcinit"][:])

            nodesT = state.tile([HID, R], f32r, tag="nodesT")

            Xnb = X_in.rearrange("b t n f -> b n t f")

            with (
                tc.tile_pool(name="xs", bufs=2) as xs_pool,
                tc.tile_pool(name="xf", bufs=2) as xf_pool,
                tc.tile_pool(name="tsb", bufs=2) as tsb_pool,
                tc.tile_pool(name="sg", bufs=2) as sg_pool,
                tc.tile_pool(name="work", bufs=2) as work,
                tc.tile_pool(name="pga", bufs=1, space=PSUM) as pga_pool,
                tc.tile_pool(name="pgb", bufs=1, space=PSUM) as pgb_pool,
                tc.tile_pool(name="pd", bufs=1, space=PSUM) as pd_pool,
                tc.tile_pool(name="pxf", bufs=2, space=PSUM) as pxf_pool,
            ):
                TRIPLES = [(0, 3), (3, 3), (6, 2)]

                def load_block(t0):
                    """DMA one TB-step X block + one-hot expand (gpsimd)."""
                    tiles = []
                    for k, (b0, nb) in enumerate(TRIPLES):
                        rows = N * nb
                        xt = xs_pool.tile([120, TB, XC], bf16,
                                          tag=f"xs{k}", name=f"xs{k}")
                        for i in range(nb):
                            nc.sync.dma_start(
                                out=xt[N * i:N * (i + 1), :, 0:F_IN],
                                in_=Xnb[b0 + i, :, t0:t0 + TB, :],
                            )
                        # merged one-hot: both categorical cols, all TB
                        # steps, all stacked graphs in one op
                        nc.vector.tensor_tensor(
                            out=xt[:rows, :, OH_P0:XC].rearrange(
                                "p t (g k) -> p t g k", k=17
                            ),
                            in0=wt["iota2b"][0:rows, :].rearrange(
                                "p (t g k) -> p t g k", t=TB, k=17
                            ),
                            in1=xt[
                                :rows, :, NUM_CONT:NUM_CONT + 2
                            ].to_broadcast([rows, TB, 2, 17]),
                            op=EQ,
                        )
                        tiles.append(xt)
                    return tiles

                def emit_trans(xtiles, tl):
                    """PE transposes -> pxf psum; returns psum tile."""
                    pxf = pxf_pool.tile([XC, R], bf16, tag="pxf")
                    for k, (b0, nb) in enumerate(TRIPLES):
                        rows = N * nb
                        nc.tensor.transpose(
                            pxf[:, 120 * k:120 * k + rows],
                            xtiles[k][:rows, tl, :],
                            wt["identb"][:rows, :rows],
                        )
                    return pxf

                def emit_tsb(t):
                    tsb = tsb_pool.tile([HID, R], bf16, tag="tsb")
                    nc.sync.dma_start(out=tsb[:], in_=tsm1_in[:, t, :])
                    return tsb

                def emit_xmm(pg, gi, g, xfT):
                    sl = slice((g % 2) * RG, (g % 2) * RG + RG)
                    nc.tensor.matmul(
                        pg[:, g // 2, sl],
                        wt["WxT"][:, HID * g:HID * (g + 1)],
                        xfT[:, GSL[gi]], start=(g % 2 == 0), stop=False,
                    )

                # ---- prologue: block 0, xfT(0), gate-x psum(0), tsb(0)
                xcur = load_block(0)
                pxf0 = emit_trans(xcur, 0)
                xfT = xf_pool.tile([XC, R], bf16, tag="xfT")
                nc.vector.tensor_scalar_add(xfT[:], pxf0[:], 0.0)
                pgA = pga_pool.tile([HID, 2, 512], f32, tag="pgA", name="pgA")
                pgB = pgb_pool.tile([HID, 2, 512], f32, tag="pgB", name="pgB")
                pgrp = [pgA, pgB]
                for gi in range(2):
                    for g in range(4):
                        emit_xmm(pgrp[gi], gi, g, xfT)
                tsb = emit_tsb(0)

                for t in range(Wsteps):
                    tl = t % TB
                    last = t == Wsteps - 1

                    # ---- c path (full width, off the critical h-chain)
                    pd = pd_pool.tile([HID, 512], f32, tag="pd")
                    nc.tensor.matmul(pd[:, 0:R], wt["WdT"][:], cT[:],
                                     start=True, stop=True)
                    cs1 = work.tile([HID, R], bf16, tag="cs1")
                    nc.scalar.activation(cs1[:], pd[:, 0:R], AF.Tanh)

                    t1 = work.tile([HID, R], bf16, tag="t1")
                    cadj = work.tile([HID, R], bf16, tag="cadj")
                    t2 = work.tile([HID, R], bf16, tag="t2")
                    t3 = work.tile([HID, R], bf16, tag="t3")
                    tnc = work.tile([HID, R], bf16, tag="tnc")
                    sg = sg_pool.tile([HID, 4, R], bf16, tag="sg")

                    for gi in range(2):
                        gsl = GSL[gi]
                        nc.vector.tensor_mul(t1[:, gsl], cs1[:, gsl],
                                             tsb[:, gsl])
                        nc.gpsimd.tensor_tensor(
                            out=cadj[:, gsl], in0=cT[0:HID, gsl],
                            in1=t1[:, gsl], op=ADD,
                        )

                    # mid-block prefetch of the next X block (gpsimd one-hot)
                    if tl == 4 and t + 4 < Wsteps:
                        xnext = load_block(t + 4)

                    tsb_n = None if last else emit_tsb(t + 1)

                    for gi in range(2):
                        gsl = GSL[gi]
                        pg = pgrp[gi]

                        # h-side accumulate onto the x-side psum
                        for g in range(4):
                            sl = slice((g % 2) * RG, (g % 2) * RG + RG)
                            nc.tensor.matmul(
                                pg[:, g // 2, sl],
                                wt["WallT"][:, HID * g:HID * (g + 1)],
                                hT[:, gsl], start=False, stop=(g % 2 == 1),
                            )

                        # one sigmoid instruction for all 4 gates of group
                        nc.scalar.activation(
                            sg[:, :, gsl].rearrange("p (b s) c -> p b s c", s=2),
                            pg[:, :, 0:2 * RG].rearrange(
                                "p b (s c) -> p b s c", c=RG
                            ),
                            AF.Sigmoid,
                        )

                        # state update: c' = f*cadj + i*ct ; h' = o*tanh(c')
                        # sg dim1: 0=f, 1=i, 2=o, 3=ct
                        f_g = sg[:, 0, gsl]
                        i_g = sg[:, 1, gsl]
                        o_g = sg[:, 2, gsl]
                        ct_g = sg[:, 3, gsl]
                        nc.vector.tensor_mul(t3[:, gsl], i_g, ct_g)
                        nc.vector.tensor_mul(t2[:, gsl], f_g, cadj[:, gsl])
                        nc.vector.tensor_add(cT[0:HID, gsl], t2[:, gsl],
                                             t3[:, gsl])

                        # next step's inputs, emitted right behind this
                        # group's sigmoid so they fill the engine pipelines:
                        # x-side matmuls for t+1 reuse this group's freed pg
                        # bank; the shared transposes/copy ride with group A.
                        if not last:
                            if gi == 0:
                                if tl == TB - 1:
                                    xcur = xnext
                                pxf = emit_trans(xcur, (t + 1) % TB)
                                xfT = xf_pool.tile([XC, R], bf16, tag="xfT")
                                nc.vector.tensor_scalar_add(
                                    xfT[:], pxf[:], 0.0)
                            for g in range(4):
                                emit_xmm(pg, gi, g, xfT)

                        nc.scalar.activation(tnc[:, gsl], cT[0:HID, gsl],
                                             AF.Tanh)
                        nc.vector.tensor_mul(hT[0:HID, gsl], o_g, tnc[:, gsl])

                    tsb = tsb_n

                # ---- output linear: nodes = relu(lin @ h + lb)
                pl = pd_pool.tile([HID, 512], f32, tag="pd")
                nc.tensor.matmul(pl[:, 0:R], wt["linT"][:], hT[:],
                                 start=True, stop=True)
                nc.scalar.activation(nodesT[:], pl[:, 0:R], AF.Relu)

            # ---------------- GCN: two SAGE layers + output proj
            with (
                tc.tile_pool(name="gc", bufs=2) as gc,
                tc.tile_pool(name="gcs", bufs=1) as gcs,
                tc.tile_pool(name="gp", bufs=2, space=PSUM) as gp,
                tc.tile_pool(name="gp1", bufs=1, space=PSUM) as gp1,
            ):
                def mean_agg(srcT, hid):
                    """srcT: [hid, R] feature-major -> aggT [hid, R]."""
                    aggT = gcs.tile([hid, R], f32r, tag=f"agg{hid}", name="aggT")
                    for b in range(BL):
                        cols = srcT[:, N * b:N * (b + 1)]   # [hid, 40] graph b
                        ptr = gp.tile([N, 128], f32, tag="ptr")
                        nc.tensor.transpose(
                            r(ptr[:, 0:hid]), cols, wt["ident"][:hid, :hid]
                        )
                        nbm = gc.tile([N, 128], f32r, tag="nbm")
                        nc.any.tensor_copy(out=nbm[:, 0:hid], in_=ptr[:, 0:hid])
                        pa = gp.tile([128, N], f32, tag="pa")
                        nc.tensor.matmul(
                            pa[0:hid, :], nbm[:, 0:hid], wt["Mmat"][:],
                            start=True, stop=True,
                        )
                        nc.any.tensor_copy(
                            out=aggT[:, N * b:N * (b + 1)], in_=pa[0:hid, :]
                        )
                    return aggT

                agg1 = mean_agg(nodesT, HID)
                pg1 = gp1.tile([64, R], f32, tag="pg1")
                nc.tensor.matmul(pg1, wt["s1l"][:], agg1[:], start=True, stop=False)
                nc.tensor.matmul(pg1, wt["s1r"][:], nodesT[:], start=False, stop=True)
                g1T = gcs.tile([64, R], f32r, tag="g1T")
                nc.scalar.activation(g1T[:], pg1, AF.Relu, bias=wt["s1b"][:].bitcast(f32))

                agg2 = mean_agg(g1T, 64)
                pg2 = gp1.tile([32, R], f32, tag="pg2")
                nc.tensor.matmul(pg2, wt["s2l"][:], agg2[:], start=True, stop=False)
                nc.tensor.matmul(pg2, wt["s2r"][:], g1T[:], start=False, stop=True)
                g2T = gcs.tile([32, R], f32r, tag="g2T")
                nc.scalar.activation(g2T[:], pg2, AF.Relu, bias=wt["s2b"][:].bitcast(f32))

                po = gp1.tile([2, R], f32, tag="po")
                nc.tensor.matmul(po, wt["ow"][:], g2T[:], start=True, stop=True)
                oT = gcs.tile([2, R], f32, tag="oT")
                nc.scalar.activation(oT[:], po, AF.Relu, bias=wt["ob"][:].bitcast(f32))

                nc.sync.dma_start(
                    out=out_ext.rearrange("k b n -> k (b n)"), in_=oT[:]
                )

    nc.compile()
    return nc


# ---------------------------------------------------------------- execution
_CACHE = {}


def _get_module():
    if "nc" not in _CACHE:
        _CACHE["nc"] = build_module()
    return _CACHE["nc"]


def make_in_maps(inputs):
    f32 = np.float32
    import ml_dtypes
    bf = ml_dtypes.bfloat16
    X = np.ascontiguousarray(np.asarray(inputs["X"], f32).astype(bf))
    ts = np.asarray(inputs["ts_list"], f32)
    wts = _host_weights(inputs)
    in_maps = []
    for c in range(NCORES):
        tsl = ts[c * BL:(c + 1) * BL]                       # [BL, W, N]
        tsm1 = (tsl.transpose(1, 0, 2).reshape(W, R) - 1.0).astype(bf)
        tsm1_rep = np.ascontiguousarray(
            np.broadcast_to(tsm1[None], (HID, W, R))
        )
        m = {"X": X[c * BL:(c + 1) * BL], "tsm1": tsm1_rep}
        m.update(wts)
        in_maps.append(m)
    return in_maps


def kernel(**inputs) -> np.ndarray:
    from concourse.bass_utils import run_bass_kernel_spmd

    nc = _get_module()
    in_maps = make_in_maps(inputs)
    res = run_bass_kernel_spmd(nc, in_maps, list(range(NCORES)))
    outs = [
        np.transpose(res.results[c]["out"], (1, 2, 0)) for c in range(NCORES)
    ]
    return np.ascontiguousarray(np.concatenate(outs, axis=0).astype(np.float32))


# revision 9
# speedup vs baseline: 1.2480x; 1.0410x over previous
"""AgentImputer Trainium2 kernel.

Contract: kernel(**inputs) takes the FULL unsharded inputs (as produced by
reference.setup_inputs()) and returns the FULL output [64, 40, 2] float32.

Strategy: data-parallel over batch B=64 across 8 NeuronCores (8 batches /
core -> 320 folded LSTM rows per core). The 128-step TimeLSTM runs
feature-major ([hid, row] tiles); categorical embeddings fold into the
input matmul via one-hot rows; biases fold into matmuls via a constant-1
state row. The recurrent loop is software-pipelined as TWO independent
column groups (rows 0:160 / 160:320) so the serial h->gates->c->h chain of
one group overlaps engine work of the other. All elementwise state math is
bf16 (DVE 2x packed mode); sigmoid over all 4 gates of a group is a single
ACT instruction; (ts-1) is host-precomputed and DMA-broadcast per step; the
per-graph GCN (shared edge_index) is dense [40,40] mean-aggregation matmuls.
"""

import sys

import numpy as np

sys.path.insert(0, "/opt/trn_rl_repo")

# ---------------------------------------------------------------- constants
B, W, N, F_IN = 64, 128, 40, 66
HID = 100
NUM_CONT = 64
NCLS_POS, NCLS_TEAM = 16, 9
EMB_POS, EMB_TEAM = 4, 3
NCORES = 8
BL = B // NCORES          # 8 local batch elems per core
R = BL * N                # 320 rows per core; row j = 40*b_local + n
RG = R // 2               # columns per pipeline group
OH_P0 = 66                # one-hot pos cols [66:83)
OH_T0 = 83                # one-hot team cols [83:100) (entries 10..16 pad)
XC = 100                  # xs tile feature columns
G4 = 4 * HID
TB = 8                    # timestep block for X prefetch


# ---------------------------------------------------------------- host prep
def _host_weights(inputs):
    import ml_dtypes
    bf = ml_dtypes.bfloat16
    f32 = np.float32
    Uall_w = np.asarray(inputs["Uall_w"], f32)       # [400, 71]
    Uall_b = np.asarray(inputs["Uall_b"], f32)       # [400]
    Wall_w = np.asarray(inputs["Wall_w"], f32)       # [400, 100]
    Wall_b = np.asarray(inputs["Wall_b"], f32)       # [400]
    Wd_w = np.asarray(inputs["Wd_w"], f32)           # [100, 100]
    Wd_b = np.asarray(inputs["Wd_b"], f32)           # [100]
    lin_w = np.asarray(inputs["lin_w"], f32)         # [100, 100]
    lin_b = np.asarray(inputs["lin_b"], f32)         # [100]
    emb_pos = np.asarray(inputs["emb_pos"], f32)     # [16, 4]
    emb_team = np.asarray(inputs["emb_team"], f32)   # [9, 3]
    edge_index = np.asarray(inputs["edge_index"]).astype(np.int64)  # [2, E]

    # Input-side weights [100, 400]: rows 0:64 continuous features; rows
    # 64,65 (raw categorical codes riding along in the transposed tile) get
    # zero weights; rows 66:83 / 83:93 are one-hot rows with the embedding
    # tables pre-multiplied in (code 0 == missing -> zero row); 93:100 pad.
    WxT = np.zeros((XC, G4), f32)
    WxT[0:NUM_CONT] = Uall_w[:, 0:NUM_CONT].T
    pad_pos = np.vstack([np.zeros((1, EMB_POS), f32), emb_pos])    # [17, 4]
    pad_team = np.vstack([np.zeros((1, EMB_TEAM), f32), emb_team])  # [10, 3]
    WxT[OH_P0:OH_T0] = pad_pos @ Uall_w[:, NUM_CONT:NUM_CONT + EMB_POS].T
    WxT[OH_T0:OH_T0 + NCLS_TEAM + 1] = (
        pad_team @ Uall_w[:, NUM_CONT + EMB_POS:].T
    )

    # h-side weights with the full gate bias folded in as an extra row
    # (state tiles carry a constant-1 row at partition HID).
    WallT = np.concatenate([Wall_w.T, (Wall_b + Uall_b)[None, :]], 0)  # [101, 400]
    WdT = np.concatenate([Wd_w.T, Wd_b[None, :]], 0)                   # [101, 100]
    linT = np.concatenate([lin_w.T, lin_b[None, :]], 0)                # [101, 100]

    # Mean-aggregation matrix: M[s, d] = count(s->d) / max(deg(d), 1)
    src, dst = edge_index[0], edge_index[1]
    cnt = np.zeros((N, N), f32)
    np.add.at(cnt, (src, dst), 1.0)
    deg = np.maximum(cnt.sum(axis=0), 1.0)
    Mmat = cnt / deg[None, :]

    # iota rows for the merged one-hot compare: [0..16 | 0..9, -1 x7],
    # replicated for each timestep of an 8-step block
    iota2 = np.concatenate([
        np.arange(NCLS_POS + 1, dtype=f32),
        np.concatenate([np.arange(NCLS_TEAM + 1, dtype=f32),
                        -np.ones(17 - (NCLS_TEAM + 1), f32)]),
    ])
    iota2b = np.tile(iota2, TB)  # [TB*34]

    return {
        "WxT": WxT.astype(bf),
        "WallT": WallT.astype(bf),
        "WdT": WdT.astype(bf),
        "linT": linT.astype(bf),
        "Mmat": np.ascontiguousarray(Mmat, f32),
        "s1l": np.ascontiguousarray(np.asarray(inputs["sage1_l"], f32).T),   # [100, 64]
        "s1r": np.ascontiguousarray(np.asarray(inputs["sage1_r"], f32).T),   # [100, 64]
        "s1b": np.ascontiguousarray(np.asarray(inputs["sage1_lb"], f32)[:, None]),  # [64, 1]
        "s2l": np.ascontiguousarray(np.asarray(inputs["sage2_l"], f32).T),   # [64, 32]
        "s2r": np.ascontiguousarray(np.asarray(inputs["sage2_r"], f32).T),   # [64, 32]
        "s2b": np.ascontiguousarray(np.asarray(inputs["sage2_lb"], f32)[:, None]),  # [32, 1]
        "ow": np.ascontiguousarray(np.asarray(inputs["out_w"], f32).T),      # [32, 2]
        "ob": np.ascontiguousarray(np.asarray(inputs["out_b"], f32)[:, None]),      # [2, 1]
        "iota2b": np.tile(iota2b, (120, 1)).astype(bf),                      # [120, TB*34]
        "hcinit": np.concatenate(
            [np.zeros((HID, R), f32), np.ones((1, R), f32)], 0
        ).astype(bf),  # [101, R]: zero state + constant-1 bias row
        "ident": np.eye(128, dtype=f32),
        "identb": np.eye(128, dtype=bf),
    }


# ---------------------------------------------------------------- device IR
def build_module(Wsteps=W):
    import concourse.bass as bass
    import concourse.tile as tile
    from concourse import bacc, mybir

    f32 = mybir.dt.float32
    f32r = mybir.dt.float32r
    bf16 = mybir.dt.bfloat16
    AF = mybir.ActivationFunctionType
    EQ = mybir.AluOpType.is_equal
    ADD = mybir.AluOpType.add
    PSUM = bass.MemorySpace.PSUM

    def r(ap):
        return ap.bitcast(f32r)

    nc = bacc.Bacc(
        "TRN2", target_bir_lowering=False, debug=False, num_devices=NCORES
    )

    X_in = nc.declare_dram_parameter("X", [BL, W, N, F_IN], bf16, isOutput=False)
    # host-precomputed (ts-1), replicated across 100 partitions: [100, W, R]
    tsm1_in = nc.declare_dram_parameter("tsm1", [HID, W, R], bf16, isOutput=False)
    w_in = {}
    bf16_params = {"WxT", "WallT", "WdT", "linT", "iota2b", "identb", "hcinit"}
    for name, shape in [
        ("WxT", [XC, G4]), ("WallT", [HID + 1, G4]), ("WdT", [HID + 1, HID]),
        ("linT", [HID + 1, HID]), ("Mmat", [N, N]),
        ("s1l", [HID, 64]), ("s1r", [HID, 64]), ("s1b", [64, 1]),
        ("s2l", [64, 32]), ("s2r", [64, 32]), ("s2b", [32, 1]),
        ("ow", [32, 2]), ("ob", [2, 1]),
        ("iota2b", [120, TB * 34]), ("hcinit", [HID + 1, R]),
        ("ident", [128, 128]), ("identb", [128, 128]),
    ]:
        w_in[name] = nc.declare_dram_parameter(
            name, shape, bf16 if name in bf16_params else f32r, isOutput=False
        )
    # device-natural layout [k, b, n]; host transposes to [b, n, k]
    out_ext = nc.declare_dram_parameter("out", [2, BL, N], f32, isOutput=True)

    GSL = [slice(0, RG), slice(RG, R)]

    with tile.TileContext(nc) as tc:
        with (
            tc.tile_pool(name="consts", bufs=1) as consts,
            tc.tile_pool(name="state", bufs=1) as state,
        ):
            # ---- load constants / weights
            wt = {}
            for name, ext in w_in.items():
                wt[name] = consts.tile(
                    list(ext.shape), ext.dtype, tag=name, name=name
                )
                nc.gpsimd.dma_start(out=wt[name][:], in_=ext[:])

            # ---- persistent state: h/c feature-major with const-1 bias row
            hT = state.tile([HID + 1, R], bf16, tag="hT")
            cT = state.tile([HID + 1, R], bf16, tag="cT")
            nc.gpsimd.dma_start(out=hT[:], in_=w_in["hcinit"][:])
            nc.gpsimd.dma_start(out=cT[:], in_=w_in["hcinit"][:])

            nodesT = state.tile([HID, R], f32r, tag="nodesT")

            Xnb = X_in.rearrange("b t n f -> b n t f")

            with (
                tc.tile_pool(name="xs", bufs=2) as xs_pool,
                tc.tile_pool(name="xf", bufs=2) as xf_pool,
                tc.tile_pool(name="tsb", bufs=2) as tsb_pool,
                tc.tile_pool(name="sg", bufs=2) as sg_pool,
                tc.tile_pool(name="work", bufs=2) as work,
                tc.tile_pool(name="pga", bufs=1, space=PSUM) as pga_pool,
                tc.tile_pool(name="pgb", bufs=1, space=PSUM) as pgb_pool,
                tc.tile_pool(name="pd", bufs=1, space=PSUM) as pd_pool,
                tc.tile_pool(name="pxf", bufs=2, space=PSUM) as pxf_pool,
            ):
                TRIPLES = [(0, 3), (3, 3), (6, 2)]

                def load_block(t0):
                    """DMA one TB-step X block + one-hot expand (gpsimd)."""
                    tiles = []
                    for k, (b0, nb) in enumerate(TRIPLES):
                        rows = N * nb
                        xt = xs_pool.tile([120, TB, XC], bf16,
                                          tag=f"xs{k}", name=f"xs{k}")
                        for i in range(nb):
                            nc.sync.dma_start(
                                out=xt[N * i:N * (i + 1), :, 0:F_IN],
                                in_=Xnb[b0 + i, :, t0:t0 + TB, :],
                            )
                        # merged one-hot: both categorical cols, all TB
                        # steps, all stacked graphs in one op
                        nc.vector.tensor_tensor(
                            out=xt[:rows, :, OH_P0:XC].rearrange(
                                "p t (g k) -> p t g k", k=17
                            ),
                            in0=wt["iota2b"][0:rows, :].rearrange(
                                "p (t g k) -> p t g k", t=TB, k=17
                            ),
                            in1=xt[
                                :rows, :, NUM_CONT:NUM_CONT + 2
                            ].to_broadcast([rows, TB, 2, 17]),
                            op=EQ,
                        )
                        tiles.append(xt)
                    return tiles

                def emit_trans(xtiles, tl):
                    """PE transposes -> pxf psum; returns psum tile."""
                    pxf = pxf_pool.tile([XC, R], bf16, tag="pxf")
                    for k, (b0, nb) in enumerate(TRIPLES):
                        rows = N * nb
                        nc.tensor.transpose(
                            pxf[:, 120 * k:120 * k + rows],
                            xtiles[k][:rows, tl, :],
                            wt["identb"][:rows, :rows],
                        )
                    return pxf

                def emit_tsb(t):
                    tsb = tsb_pool.tile([HID, R], bf16, tag="tsb")
                    nc.sync.dma_start(out=tsb[:], in_=tsm1_in[:, t, :])
                    return tsb

                def emit_xmm(pg, gi, g, xfT):
                    sl = slice((g % 2) * RG, (g % 2) * RG + RG)
                    nc.tensor.matmul(
                        pg[:, g // 2, sl],
                        wt["WxT"][:, HID * g:HID * (g + 1)],
                        xfT[:, GSL[gi]], start=(g % 2 == 0), stop=False,
                    )

                def emit_wd(pdn):
                    nc.tensor.matmul(pdn[:, 0:R], wt["WdT"][:], cT[:],
                                     start=True, stop=True)

                # ---- prologue: block 0, xfT(0), gate-x psum(0), tsb(0)
                xcur = load_block(0)
                pxf0 = emit_trans(xcur, 0)
                xfT = xf_pool.tile([XC, R], bf16, tag="xfT")
                nc.vector.tensor_scalar_add(xfT[:], pxf0[:], 0.0)
                pgA = pga_pool.tile([HID, 2, 512], f32, tag="pgA", name="pgA")
                pgB = pgb_pool.tile([HID, 2, 512], f32, tag="pgB", name="pgB")
                pgrp = [pgA, pgB]
                for gi in range(2):
                    for g in range(4):
                        emit_xmm(pgrp[gi], gi, g, xfT)
                tsb = emit_tsb(0)
                pd = pd_pool.tile([HID, 512], f32, tag="pd")
                emit_wd(pd)

                for t in range(Wsteps):
                    tl = t % TB
                    last = t == Wsteps - 1

                    # ---- c path (off the critical h-chain): cs1=tanh(Wd@c+bd)
                    cs1 = work.tile([HID, R], bf16, tag="cs1")
                    nc.scalar.activation(cs1[:], pd[:, 0:R], AF.Tanh)

                    t1 = work.tile([HID, R], bf16, tag="t1")
                    cadj = work.tile([HID, R], bf16, tag="cadj")
                    t2 = work.tile([HID, R], bf16, tag="t2")
                    t3 = work.tile([HID, R], bf16, tag="t3")
                    tnc = work.tile([HID, R], bf16, tag="tnc")
                    sg = sg_pool.tile([HID, 4, R], bf16, tag="sg")

                    # t1/cadj per group; cadj_A on gpsimd, cadj_B on DVE so
                    # group B's c-cycle never waits behind A on the Pool queue
                    for gi in range(2):
                        gsl = GSL[gi]
                        nc.vector.tensor_mul(t1[:, gsl], cs1[:, gsl],
                                             tsb[:, gsl])
                        eng = nc.gpsimd if gi == 0 else nc.vector
                        eng.tensor_tensor(
                            out=cadj[:, gsl], in0=cT[0:HID, gsl],
                            in1=t1[:, gsl], op=ADD,
                        )

                    # mid-block prefetch of the next X block
                    if tl == 4 and t + 4 < Wsteps:
                        xnext = load_block(t + 4)

                    tsb_n = None if last else emit_tsb(t + 1)

                    # next step's xfT: transposes go behind hmm_A on PE; the
                    # psum->sbuf copy sits early in the DVE stream (it parks
                    # until the transposes land, while later DVE ops bypass).
                    if not last:
                        if tl == TB - 1:
                            xcur = xnext
                        pxf = emit_trans(xcur, (t + 1) % TB)
                        xfT = xf_pool.tile([XC, R], bf16, tag="xfT")
                        nc.vector.tensor_scalar_add(xfT[:], pxf[:], 0.0)

                    for gi in range(2):
                        gsl = GSL[gi]
                        pg = pgrp[gi]

                        # h-side accumulate onto the x-side psum
                        for g in range(4):
                            sl = slice((g % 2) * RG, (g % 2) * RG + RG)
                            nc.tensor.matmul(
                                pg[:, g // 2, sl],
                                wt["WallT"][:, HID * g:HID * (g + 1)],
                                hT[:, gsl], start=False, stop=(g % 2 == 1),
                            )

                        # one sigmoid instruction for all 4 gates of group
                        nc.scalar.activation(
                            sg[:, :, gsl].rearrange("p (b s) c -> p b s c", s=2),
                            pg[:, :, 0:2 * RG].rearrange(
                                "p b (s c) -> p b s c", c=RG
                            ),
                            AF.Sigmoid,
                        )

                        # state update: c' = f*cadj + i*ct
                        # sg dim1: 0=f, 1=i, 2=o, 3=ct
                        nc.vector.tensor_mul(t3[:, gsl], sg[:, 1, gsl],
                                             sg[:, 3, gsl])
                        nc.vector.tensor_mul(t2[:, gsl], sg[:, 0, gsl],
                                             cadj[:, gsl])
                        nc.vector.tensor_add(cT[0:HID, gsl], t2[:, gsl],
                                             t3[:, gsl])

                        # next step's x-side matmuls reuse this group's freed
                        # pg banks right after its sigmoid; Wd(t+1) follows
                        # hmm_B so it fires the moment c'_B lands.
                        if not last:
                            for g in range(4):
                                emit_xmm(pg, gi, g, xfT)
                            if gi == 1:
                                pd = pd_pool.tile([HID, 512], f32, tag="pd")
                                emit_wd(pd)

                    # ---- step tails: h' = o*tanh(c') for both groups
                    for gi in range(2):
                        gsl = GSL[gi]
                        nc.scalar.activation(tnc[:, gsl], cT[0:HID, gsl],
                                             AF.Tanh)
                        nc.vector.tensor_mul(hT[0:HID, gsl], sg[:, 2, gsl],
                                             tnc[:, gsl])

                    tsb = tsb_n

                # ---- output linear: nodes = relu(lin @ h + lb)
                pl = pd_pool.tile([HID, 512], f32, tag="pd")
                nc.tensor.matmul(pl[:, 0:R], wt["linT"][:], hT[:],
                                 start=True, stop=True)
                nc.scalar.activation(nodesT[:], pl[:, 0:R], AF.Relu)

            # ---------------- GCN: two SAGE layers + output proj
            with (
                tc.tile_pool(name="gc", bufs=2) as gc,
                tc.tile_pool(name="gcs", bufs=1) as gcs,
                tc.tile_pool(name="gp", bufs=2, space=PSUM) as gp,
                tc.tile_pool(name="gp1", bufs=1, space=PSUM) as gp1,
            ):
                def mean_agg(srcT, hid):
                    """srcT: [hid, R] feature-major -> aggT [hid, R]."""
                    aggT = gcs.tile([hid, R], f32r, tag=f"agg{hid}", name="aggT")
                    for b in range(BL):
                        cols = srcT[:, N * b:N * (b + 1)]   # [hid, 40] graph b
                        ptr = gp.tile([N, 128], f32, tag="ptr")
                        nc.tensor.transpose(
                            r(ptr[:, 0:hid]), cols, wt["ident"][:hid, :hid]
                        )
                        nbm = gc.tile([N, 128], f32r, tag="nbm")
                        nc.any.tensor_copy(out=nbm[:, 0:hid], in_=ptr[:, 0:hid])
                        pa = gp.tile([128, N], f32, tag="pa")
                        nc.tensor.matmul(
                            pa[0:hid, :], nbm[:, 0:hid], wt["Mmat"][:],
                            start=True, stop=True,
                        )
                        nc.any.tensor_copy(
                            out=aggT[:, N * b:N * (b + 1)], in_=pa[0:hid, :]
                        )
                    return aggT

                agg1 = mean_agg(nodesT, HID)
                pg1 = gp1.tile([64, R], f32, tag="pg1")
                nc.tensor.matmul(pg1, wt["s1l"][:], agg1[:], start=True, stop=False)
                nc.tensor.matmul(pg1, wt["s1r"][:], nodesT[:], start=False, stop=True)
                g1T = gcs.tile([64, R], f32r, tag="g1T")
                nc.scalar.activation(g1T[:], pg1, AF.Relu, bias=wt["s1b"][:].bitcast(f32))

                agg2 = mean_agg(g1T, 64)
                pg2 = gp1.tile([32, R], f32, tag="pg2")
                nc.tensor.matmul(pg2, wt["s2l"][:], agg2[:], start=True, stop=False)
                nc.tensor.matmul(pg2, wt["s2r"][:], g1T[:], start=False, stop=True)
                g2T = gcs.tile([32, R], f32r, tag="g2T")
                nc.scalar.activation(g2T[:], pg2, AF.Relu, bias=wt["s2b"][:].bitcast(f32))

                po = gp1.tile([2, R], f32, tag="po")
                nc.tensor.matmul(po, wt["ow"][:], g2T[:], start=True, stop=True)
                oT = gcs.tile([2, R], f32, tag="oT")
                nc.scalar.activation(oT[:], po, AF.Relu, bias=wt["ob"][:].bitcast(f32))

                nc.sync.dma_start(
                    out=out_ext.rearrange("k b n -> k (b n)"), in_=oT[:]
                )

    nc.compile()
    return nc


# ---------------------------------------------------------------- execution
_CACHE = {}


def _get_module():
    if "nc" not in _CACHE:
        _CACHE["nc"] = build_module()
    return _CACHE["nc"]


def make_in_maps(inputs):
    f32 = np.float32
    import ml_dtypes
    bf = ml_dtypes.bfloat16
    X = np.ascontiguousarray(np.asarray(inputs["X"], f32).astype(bf))
    ts = np.asarray(inputs["ts_list"], f32)
    wts = _host_weights(inputs)
    in_maps = []
    for c in range(NCORES):
        tsl = ts[c * BL:(c + 1) * BL]                       # [BL, W, N]
        tsm1 = (tsl.transpose(1, 0, 2).reshape(W, R) - 1.0).astype(bf)
        tsm1_rep = np.ascontiguousarray(
            np.broadcast_to(tsm1[None], (HID, W, R))
        )
        m = {"X": X[c * BL:(c + 1) * BL], "tsm1": tsm1_rep}
        m.update(wts)
        in_maps.append(m)
    return in_maps


def kernel(**inputs) -> np.ndarray:
    from concourse.bass_utils import run_bass_kernel_spmd

    nc = _get_module()
    in_maps = make_in_maps(inputs)
    res = run_bass_kernel_spmd(nc, in_maps, list(range(NCORES)))
    outs = [
        np.transpose(res.results[c]["out"], (1, 2, 0)) for c in range(NCORES)
    ]
    return np.ascontiguousarray(np.concatenate(outs, axis=0).astype(np.float32))


# revision 17
# speedup vs baseline: 1.3034x; 1.0444x over previous
"""AgentImputer Trainium2 kernel.

Contract: kernel(**inputs) takes the FULL unsharded inputs (as produced by
reference.setup_inputs()) and returns the FULL output [64, 40, 2] float32.

Strategy: data-parallel over batch B=64 across 8 NeuronCores (8 batches /
core -> 320 folded LSTM rows per core). The 128-step TimeLSTM runs
feature-major ([hid, row] tiles); categorical embeddings fold into the
input matmul via one-hot rows; biases fold into matmuls via a constant-1
state row. The recurrent loop is software-pipelined as TWO independent
column groups (rows 0:160 / 160:320) so the serial h->gates->c->h chain of
one group overlaps engine work of the other. All elementwise state math is
bf16 (DVE 2x packed mode); sigmoid over all 4 gates of a group is a single
ACT instruction; (ts-1) is host-precomputed and DMA-broadcast per step; the
per-graph GCN (shared edge_index) is dense [40,40] mean-aggregation matmuls.
"""

import sys

import numpy as np

sys.path.insert(0, "/opt/trn_rl_repo")

# ---------------------------------------------------------------- constants
B, W, N, F_IN = 64, 128, 40, 66
HID = 100
NUM_CONT = 64
NCLS_POS, NCLS_TEAM = 16, 9
EMB_POS, EMB_TEAM = 4, 3
NCORES = 8
BL = B // NCORES          # 8 local batch elems per core
R = BL * N                # 320 rows per core; row j = 40*b_local + n
RG = R // 2               # columns per pipeline group
OH_P0 = 66                # one-hot pos cols [66:83)
OH_T0 = 83                # one-hot team cols [83:100) (entries 10..16 pad)
XC = 100                  # xs tile feature columns
G4 = 4 * HID
TB = 8                    # timestep block for X prefetch


# ---------------------------------------------------------------- host prep
def _host_weights(inputs):
    import ml_dtypes
    bf = ml_dtypes.bfloat16
    f32 = np.float32
    Uall_w = np.asarray(inputs["Uall_w"], f32)       # [400, 71]
    Uall_b = np.asarray(inputs["Uall_b"], f32)       # [400]
    Wall_w = np.asarray(inputs["Wall_w"], f32)       # [400, 100]
    Wall_b = np.asarray(inputs["Wall_b"], f32)       # [400]
    Wd_w = np.asarray(inputs["Wd_w"], f32)           # [100, 100]
    Wd_b = np.asarray(inputs["Wd_b"], f32)           # [100]
    lin_w = np.asarray(inputs["lin_w"], f32)         # [100, 100]
    lin_b = np.asarray(inputs["lin_b"], f32)         # [100]
    emb_pos = np.asarray(inputs["emb_pos"], f32)     # [16, 4]
    emb_team = np.asarray(inputs["emb_team"], f32)   # [9, 3]
    edge_index = np.asarray(inputs["edge_index"]).astype(np.int64)  # [2, E]

    # Input-side weights [100, 400]: rows 0:64 continuous features; rows
    # 64,65 (raw categorical codes riding along in the transposed tile) get
    # zero weights; rows 66:83 / 83:93 are one-hot rows with the embedding
    # tables pre-multiplied in (code 0 == missing -> zero row); 93:100 pad.
    WxT = np.zeros((XC, G4), f32)
    WxT[0:NUM_CONT] = Uall_w[:, 0:NUM_CONT].T
    pad_pos = np.vstack([np.zeros((1, EMB_POS), f32), emb_pos])    # [17, 4]
    pad_team = np.vstack([np.zeros((1, EMB_TEAM), f32), emb_team])  # [10, 3]
    WxT[OH_P0:OH_T0] = pad_pos @ Uall_w[:, NUM_CONT:NUM_CONT + EMB_POS].T
    WxT[OH_T0:OH_T0 + NCLS_TEAM + 1] = (
        pad_team @ Uall_w[:, NUM_CONT + EMB_POS:].T
    )

    # h-side weights with the full gate bias folded in as an extra row
    # (state tiles carry a constant-1 row at partition HID).
    WallT = np.concatenate([Wall_w.T, (Wall_b + Uall_b)[None, :]], 0)  # [101, 400]
    WdT = np.concatenate([Wd_w.T, Wd_b[None, :]], 0)                   # [101, 100]
    linT = np.concatenate([lin_w.T, lin_b[None, :]], 0)                # [101, 100]

    # Mean-aggregation matrix: M[s, d] = count(s->d) / max(deg(d), 1)
    src, dst = edge_index[0], edge_index[1]
    cnt = np.zeros((N, N), f32)
    np.add.at(cnt, (src, dst), 1.0)
    deg = np.maximum(cnt.sum(axis=0), 1.0)
    Mmat = cnt / deg[None, :]

    # iota rows for the merged one-hot compare: [0..16 | 0..9, -1 x7],
    # replicated for each timestep of an 8-step block
    iota2 = np.concatenate([
        np.arange(NCLS_POS + 1, dtype=f32),
        np.concatenate([np.arange(NCLS_TEAM + 1, dtype=f32),
                        -np.ones(17 - (NCLS_TEAM + 1), f32)]),
    ])
    iota2b = np.tile(iota2, TB)  # [TB*34]

    return {
        "WxT": WxT.astype(bf),
        "WallT": WallT.astype(bf),
        "WdT": WdT.astype(bf),
        "linT": linT.astype(bf),
        "Mmat": np.ascontiguousarray(Mmat, f32),
        "s1l": np.ascontiguousarray(np.asarray(inputs["sage1_l"], f32).T),   # [100, 64]
        "s1r": np.ascontiguousarray(np.asarray(inputs["sage1_r"], f32).T),   # [100, 64]
        "s1b": np.ascontiguousarray(np.asarray(inputs["sage1_lb"], f32)[:, None]),  # [64, 1]
        "s2l": np.ascontiguousarray(np.asarray(inputs["sage2_l"], f32).T),   # [64, 32]
        "s2r": np.ascontiguousarray(np.asarray(inputs["sage2_r"], f32).T),   # [64, 32]
        "s2b": np.ascontiguousarray(np.asarray(inputs["sage2_lb"], f32)[:, None]),  # [32, 1]
        "ow": np.ascontiguousarray(np.asarray(inputs["out_w"], f32).T),      # [32, 2]
        "ob": np.ascontiguousarray(np.asarray(inputs["out_b"], f32)[:, None]),      # [2, 1]
        "iota2b": np.tile(iota2b, (120, 1)).astype(bf),                      # [120, TB*34]
        "hcinit": np.concatenate(
            [np.zeros((HID, R), f32), np.ones((1, R), f32)], 0
        ).astype(bf),  # [101, R]: zero state + constant-1 bias row
        "ident": np.eye(128, dtype=f32),
        "identb": np.eye(128, dtype=bf),
    }


# ---------------------------------------------------------------- device IR
def build_module(Wsteps=W):
    import concourse.bass as bass
    import concourse.tile as tile
    from concourse import bacc, mybir

    f32 = mybir.dt.float32
    f32r = mybir.dt.float32r
    bf16 = mybir.dt.bfloat16
    AF = mybir.ActivationFunctionType
    EQ = mybir.AluOpType.is_equal
    ADD = mybir.AluOpType.add
    PSUM = bass.MemorySpace.PSUM

    def r(ap):
        return ap.bitcast(f32r)

    nc = bacc.Bacc(
        "TRN2", target_bir_lowering=False, debug=False, num_devices=NCORES
    )

    X_in = nc.declare_dram_parameter("X", [BL, W, N, F_IN], bf16, isOutput=False)
    # host-precomputed (ts-1), replicated across 100 partitions: [100, W, R]
    tsm1_in = nc.declare_dram_parameter("tsm1", [HID, W, R], bf16, isOutput=False)
    w_in = {}
    bf16_params = {"WxT", "WallT", "WdT", "linT", "iota2b", "identb", "hcinit"}
    for name, shape in [
        ("WxT", [XC, G4]), ("WallT", [HID + 1, G4]), ("WdT", [HID + 1, HID]),
        ("linT", [HID + 1, HID]), ("Mmat", [N, N]),
        ("s1l", [HID, 64]), ("s1r", [HID, 64]), ("s1b", [64, 1]),
        ("s2l", [64, 32]), ("s2r", [64, 32]), ("s2b", [32, 1]),
        ("ow", [32, 2]), ("ob", [2, 1]),
        ("iota2b", [120, TB * 34]), ("hcinit", [HID + 1, R]),
        ("ident", [128, 128]), ("identb", [128, 128]),
    ]:
        w_in[name] = nc.declare_dram_parameter(
            name, shape, bf16 if name in bf16_params else f32r, isOutput=False
        )
    # device-natural layout [k, b, n]; host transposes to [b, n, k]
    out_ext = nc.declare_dram_parameter("out", [2, BL, N], f32, isOutput=True)

    GSL = [slice(0, RG), slice(RG, R)]

    with tile.TileContext(nc) as tc:
        with (
            tc.tile_pool(name="consts", bufs=1) as consts,
            tc.tile_pool(name="state", bufs=1) as state,
        ):
            # ---- load constants / weights
            wt = {}
            for name, ext in w_in.items():
                wt[name] = consts.tile(
                    list(ext.shape), ext.dtype, tag=name, name=name
                )
                nc.gpsimd.dma_start(out=wt[name][:], in_=ext[:])

            # ---- persistent state: h/c feature-major with const-1 bias row
            hT = state.tile([HID + 1, R], bf16, tag="hT")
            cT = state.tile([HID + 1, R], bf16, tag="cT")
            nc.gpsimd.dma_start(out=hT[:], in_=w_in["hcinit"][:])
            nc.gpsimd.dma_start(out=cT[:], in_=w_in["hcinit"][:])

            nodesT = state.tile([HID, R], f32r, tag="nodesT")

            Xnb = X_in.rearrange("b t n f -> b n t f")

            with (
                tc.tile_pool(name="xs", bufs=2) as xs_pool,
                tc.tile_pool(name="xf", bufs=2) as xf_pool,
                tc.tile_pool(name="tsb", bufs=2) as tsb_pool,
                tc.tile_pool(name="sg", bufs=2) as sg_pool,
                tc.tile_pool(name="work", bufs=2) as work,
                tc.tile_pool(name="pga", bufs=1, space=PSUM) as pga_pool,
                tc.tile_pool(name="pgb", bufs=1, space=PSUM) as pgb_pool,
                tc.tile_pool(name="pd", bufs=2, space=PSUM) as pd_pool,
                tc.tile_pool(name="pxf", bufs=2, space=PSUM) as pxf_pool,
            ):
                TRIPLES = [(0, 3), (3, 3), (6, 2)]

                def load_block(t0):
                    """DMA one TB-step X block (one-hots emitted separately)."""
                    tiles = []
                    for k, (b0, nb) in enumerate(TRIPLES):
                        xt = xs_pool.tile([120, TB, XC], bf16,
                                          tag=f"xs{k}", name=f"xs{k}")
                        for i in range(nb):
                            nc.sync.dma_start(
                                out=xt[N * i:N * (i + 1), :, 0:F_IN],
                                in_=Xnb[b0 + i, :, t0:t0 + TB, :],
                            )
                        tiles.append(xt)
                    return tiles

                def emit_onehot(xtiles, k):
                    # merged one-hot for triple k: both categorical cols,
                    # all TB steps, all stacked graphs in one op
                    rows = N * TRIPLES[k][1]
                    xt = xtiles[k]
                    nc.vector.tensor_tensor(
                        out=xt[:rows, :, OH_P0:XC].rearrange(
                            "p t (g k) -> p t g k", k=17
                        ),
                        in0=wt["iota2b"][0:rows, :].rearrange(
                            "p (t g k) -> p t g k", t=TB, k=17
                        ),
                        in1=xt[
                            :rows, :, NUM_CONT:NUM_CONT + 2
                        ].to_broadcast([rows, TB, 2, 17]),
                        op=EQ,
                    )

                def emit_trans(xtiles, tl):
                    """PE transposes -> pxf psum; returns psum tile."""
                    pxf = pxf_pool.tile([XC, R], bf16, tag="pxf")
                    for k, (b0, nb) in enumerate(TRIPLES):
                        rows = N * nb
                        nc.tensor.transpose(
                            pxf[:, 120 * k:120 * k + rows],
                            xtiles[k][:rows, tl, :],
                            wt["identb"][:rows, :rows],
                        )
                    return pxf

                def emit_tsb(t):
                    tsb = tsb_pool.tile([HID, R], bf16, tag="tsb")
                    nc.sync.dma_start(out=tsb[:], in_=tsm1_in[:, t, :])
                    return tsb

                def emit_xmm(pg, gi, g, xfT):
                    sl = slice((g % 2) * RG, (g % 2) * RG + RG)
                    nc.tensor.matmul(
                        pg[:, g // 2, sl],
                        wt["WxT"][:, HID * g:HID * (g + 1)],
                        xfT[:, GSL[gi]], start=(g % 2 == 0), stop=False,
                    )

                def emit_wd(pdn):
                    nc.tensor.matmul(pdn[:, 0:R], wt["WdT"][:], cT[:],
                                     start=True, stop=True)

                # ---- prologue: block 0, xfT(0), gate-x psum(0), tsb(0)
                xcur = load_block(0)
                for k in range(3):
                    emit_onehot(xcur, k)
                pxf0 = emit_trans(xcur, 0)
                xfT = xf_pool.tile([XC, R], bf16, tag="xfT")
                nc.vector.tensor_scalar_add(xfT[:], pxf0[:], 0.0)
                pgA = pga_pool.tile([HID, 2, 512], f32, tag="pgA", name="pgA")
                pgB = pgb_pool.tile([HID, 2, 512], f32, tag="pgB", name="pgB")
                pgrp = [pgA, pgB]
                for gi in range(2):
                    for g in range(4):
                        emit_xmm(pgrp[gi], gi, g, xfT)
                tsb = emit_tsb(0)
                pd = pd_pool.tile([HID, 512], f32, tag="pd")
                emit_wd(pd)
                xnext_fresh = False

                for t in range(Wsteps):
                    tl = t % TB
                    last = t == Wsteps - 1

                    # ---- c path (off the critical h-chain): cs1=tanh(Wd@c+bd)
                    cs1 = work.tile([HID, R], bf16, tag="cs1")
                    nc.scalar.activation(cs1[:], pd[:, 0:R], AF.Tanh)

                    t1 = work.tile([HID, R], bf16, tag="t1")
                    cadj = work.tile([HID, R], bf16, tag="cadj")
                    t2 = work.tile([HID, R], bf16, tag="t2")
                    t3 = work.tile([HID, R], bf16, tag="t3")
                    tnc = work.tile([HID, R], bf16, tag="tnc")
                    sg = sg_pool.tile([HID, 4, R], bf16, tag="sg")

                    # t1/cadj as full-width single ops (fewer instruction
                    # overheads; the c-path is full-width-coupled anyway)
                    nc.vector.tensor_mul(t1[:], cs1[:], tsb[:])
                    nc.vector.tensor_add(cadj[:], cT[0:HID, :], t1[:])

                    # mid-block prefetch of the next X block
                    if tl == 4 and t + 4 < Wsteps:
                        xnext = load_block(t + 4)
                        xnext_fresh = True

                    tsb_n = None if last else emit_tsb(t + 1)

                    # next step's xfT: transposes go behind hmm_A on PE; the
                    # psum->sbuf copy sits early in the DVE stream (it parks
                    # until the transposes land, while later DVE ops bypass).
                    if not last:
                        if tl == TB - 1:
                            xcur = xnext
                        pxf = emit_trans(xcur, (t + 1) % TB)
                        xfT = xf_pool.tile([XC, R], bf16, tag="xfT")
                        nc.vector.tensor_scalar_add(xfT[:], pxf[:], 0.0)

                    for gi in range(2):
                        gsl = GSL[gi]
                        pg = pgrp[gi]

                        # h-side accumulate onto the x-side psum
                        for g in range(4):
                            sl = slice((g % 2) * RG, (g % 2) * RG + RG)
                            nc.tensor.matmul(
                                pg[:, g // 2, sl],
                                wt["WallT"][:, HID * g:HID * (g + 1)],
                                hT[:, gsl], start=False, stop=(g % 2 == 1),
                            )

                        # one sigmoid instruction for all 4 gates of group
                        nc.scalar.activation(
                            sg[:, :, gsl].rearrange("p (b s) c -> p b s c", s=2),
                            pg[:, :, 0:2 * RG].rearrange(
                                "p b (s c) -> p b s c", c=RG
                            ),
                            AF.Sigmoid,
                        )

                        # state update: c' = f*cadj + i*ct
                        # sg dim1: 0=f, 1=i, 2=o, 3=ct
                        nc.vector.tensor_mul(t3[:, gsl], sg[:, 1, gsl],
                                             sg[:, 3, gsl])
                        nc.vector.tensor_mul(t2[:, gsl], sg[:, 0, gsl],
                                             cadj[:, gsl])
                        nc.vector.tensor_add(cT[0:HID, gsl], t2[:, gsl],
                                             t3[:, gsl])

                        # next step's x-side matmuls reuse this group's freed
                        # pg banks right after its sigmoid; Wd(t+1) parks in
                        # the PE wait queue and fires the moment c'_B lands.
                        if not last:
                            if gi == 1:
                                pd = pd_pool.tile([HID, 512], f32, tag="pd")
                                emit_wd(pd)
                            for g in range(4):
                                emit_xmm(pg, gi, g, xfT)

                    # ---- step tails: h' = o*tanh(c') for both groups
                    for gi in range(2):
                        gsl = GSL[gi]
                        nc.scalar.activation(tnc[:, gsl], cT[0:HID, gsl],
                                             AF.Tanh)
                        nc.vector.tensor_mul(hT[0:HID, gsl], sg[:, 2, gsl],
                                             tnc[:, gsl])

                    # one-hot expansion for the prefetched block rides in the
                    # DVE lull at step tails (one triple per step)
                    if xnext_fresh and tl in (4, 5, 6):
                        emit_onehot(xnext, tl - 4)
                        if tl == 6:
                            xnext_fresh = False

                    tsb = tsb_n

                # ---- output linear: nodes = relu(lin @ h + lb)
                pl = pd_pool.tile([HID, 512], f32, tag="pd")
                nc.tensor.matmul(pl[:, 0:R], wt["linT"][:], hT[:],
                                 start=True, stop=True)
                nc.scalar.activation(nodesT[:], pl[:, 0:R], AF.Relu)

            # ---------------- GCN: two SAGE layers + output proj
            with (
                tc.tile_pool(name="gc", bufs=2) as gc,
                tc.tile_pool(name="gcs", bufs=1) as gcs,
                tc.tile_pool(name="gp", bufs=2, space=PSUM) as gp,
                tc.tile_pool(name="gp1", bufs=1, space=PSUM) as gp1,
            ):
                def mean_agg(srcT, hid):
                    """srcT: [hid, R] feature-major -> aggT [hid, R]."""
                    aggT = gcs.tile([hid, R], f32r, tag=f"agg{hid}", name="aggT")
                    for b in range(BL):
                        cols = srcT[:, N * b:N * (b + 1)]   # [hid, 40] graph b
                        ptr = gp.tile([N, 128], f32, tag="ptr")
                        nc.tensor.transpose(
                            r(ptr[:, 0:hid]), cols, wt["ident"][:hid, :hid]
                        )
                        nbm = gc.tile([N, 128], f32r, tag="nbm")
                        nc.any.tensor_copy(out=nbm[:, 0:hid], in_=ptr[:, 0:hid])
                        pa = gp.tile([128, N], f32, tag="pa")
                        nc.tensor.matmul(
                            pa[0:hid, :], nbm[:, 0:hid], wt["Mmat"][:],
                            start=True, stop=True,
                        )
                        nc.any.tensor_copy(
                            out=aggT[:, N * b:N * (b + 1)], in_=pa[0:hid, :]
                        )
                    return aggT

                agg1 = mean_agg(nodesT, HID)
                pg1 = gp1.tile([64, R], f32, tag="pg1")
                nc.tensor.matmul(pg1, wt["s1l"][:], agg1[:], start=True, stop=False)
                nc.tensor.matmul(pg1, wt["s1r"][:], nodesT[:], start=False, stop=True)
                g1T = gcs.tile([64, R], f32r, tag="g1T")
                nc.scalar.activation(g1T[:], pg1, AF.Relu, bias=wt["s1b"][:].bitcast(f32))

                agg2 = mean_agg(g1T, 64)
                pg2 = gp1.tile([32, R], f32, tag="pg2")
                nc.tensor.matmul(pg2, wt["s2l"][:], agg2[:], start=True, stop=False)
                nc.tensor.matmul(pg2, wt["s2r"][:], g1T[:], start=False, stop=True)
                g2T = gcs.tile([32, R], f32r, tag="g2T")
                nc.scalar.activation(g2T[:], pg2, AF.Relu, bias=wt["s2b"][:].bitcast(f32))

                po = gp1.tile([2, R], f32, tag="po")
                nc.tensor.matmul(po, wt["ow"][:], g2T[:], start=True, stop=True)
                oT = gcs.tile([2, R], f32, tag="oT")
                nc.scalar.activation(oT[:], po, AF.Relu, bias=wt["ob"][:].bitcast(f32))

                nc.sync.dma_start(
                    out=out_ext.rearrange("k b n -> k (b n)"), in_=oT[:]
                )

    nc.compile()
    return nc


# ---------------------------------------------------------------- execution
_CACHE = {}


def _get_module():
    if "nc" not in _CACHE:
        _CACHE["nc"] = build_module()
    return _CACHE["nc"]


def make_in_maps(inputs):
    f32 = np.float32
    import ml_dtypes
    bf = ml_dtypes.bfloat16
    X = np.ascontiguousarray(np.asarray(inputs["X"], f32).astype(bf))
    ts = np.asarray(inputs["ts_list"], f32)
    wts = _host_weights(inputs)
    in_maps = []
    for c in range(NCORES):
        tsl = ts[c * BL:(c + 1) * BL]                       # [BL, W, N]
        tsm1 = (tsl.transpose(1, 0, 2).reshape(W, R) - 1.0).astype(bf)
        tsm1_rep = np.ascontiguousarray(
            np.broadcast_to(tsm1[None], (HID, W, R))
        )
        m = {"X": X[c * BL:(c + 1) * BL], "tsm1": tsm1_rep}
        m.update(wts)
        in_maps.append(m)
    return in_maps


def kernel(**inputs) -> np.ndarray:
    from concourse.bass_utils import run_bass_kernel_spmd

    nc = _get_module()
    in_maps = make_in_maps(inputs)
    res = run_bass_kernel_spmd(nc, in_maps, list(range(NCORES)))
    outs = [
        np.transpose(res.results[c]["out"], (1, 2, 0)) for c in range(NCORES)
    ]
    return np.ascontiguousarray(np.concatenate(outs, axis=0).astype(np.float32))
